# revision 54
# baseline (speedup 1.0000x reference)
"""Trainium2 Bass kernel for nn_DeformableAlignment.

Sharding: 8 cores = (batch b in 0..4) x (image row-half in {0,1}).
Each core computes out[b, :, y0:y0+64, :] for y0 = 64*(i%2).

Math (per core, matches reference exactly):
  om  = conv3x3(f1-half on device) + omf (f3-half, host) [27, 64, 128]
  dy/dx per tap k; sg = sigmoid(mask-channels)
  bilinear warp written floor-free via hat fields at the OUTPUT pixel:
    Bf[k,sx,sy] = relu(1-|dx-sx|) * relu(1-|dy-sy|)*sg  (sx,sy in -2..2)
  g[k] = 1x1-conv of f1 with main_w tap k, computed ONCE on the
         y-padded grid: g[x=128 part, 9k, 64o, 70y]     (140 matmuls)
  acc[x,o,y] = sum_{k,sx,sy} Bf[x,k,sx,sy,y] * g[x+dlt, k, o, y+ky+sy],
         dlt = kx-1+sx: x-shifts of g are 12 partition-shifted
         SBUF->SBUF DMAs (per dlt & kx plane); the 5 sy taps are fused
         per op via a sliding-window AP + reduce.
  BN stats via on-device partial sums + AllReduce across 8 cores; the
  BN affine is pre-divided by OUT_SCALE so the output quantizes to int8.

IO is minimized for the axon tunnel (aggregate ~45-70MB/s shared by
both directions, ~47ms dispatch RTT that pipelines away, and ~50ms of
NON-pipelined fixed cost per device_put -> everything rides ONE tensor):
  dat   [64, 13056] int8, merged per-call upload:
        cols 0:8960     f1 rows y0-3..y0+66 as [70,128], quantized at
                        4/127 (clip 4sig; scale folded into ow/wk
                        host-side, so ints convert straight to bf16)
        cols 8960:13056 omf = the f3 half of the offset conv (+bias),
                        computed host-side (f3 only feeds this
                        27-channel conv, so shipping the contracted
                        result cuts that upload 64ch -> 27ch); channel
                        q's 8192B split across partitions 2q/2q+1,
                        un-permuted on device by one affine DMA;
                        per-channel scales ride in spack col 2.
  wpack [128, 659] bf16: ow_t f1-half [64,243] | wk packed [128,288]
        | ident (ow/wk scaled by the f1 quant step)
  spack [128, 4] f32: sel | sel | omf scales | (gamma,beta)/OUT_SCALE
  out   [64, 64, 128] int8 (dequantized to f32 on host; BN output is
        ~N(0,1) per channel so a 4/127 step keeps rel err ~1.75%)

Runner: under axon, run_bass_kernel_spmd would rebuild a jax.jit
(re-trace + re-lower, embedding the multi-MB BIR) and re-upload
weights + donated zero output buffers on EVERY call.  _AxonRunner
instead keeps one persistent shard_map jit, keeps wpack/spack resident
on device (re-uploaded only if the weight bytes change), and donates
the PREVIOUS call's output buffer as the next call's scratch (the
kernel writes every output element, so the scratch contents are
irrelevant after the first call's zeros).  Per call the tunnel moves
one 6.7MB upload and one 4.2MB download.  kernel() self-checks the BN
output invariant (per-channel mean==beta, std==|gamma|) and retries
the rare corrupted round; on the sporadic hard device crash
(NRT_EXEC_UNIT_UNRECOVERABLE poisons the process's PJRT client) it
recomputes in a fresh subprocess.
"""

import os

# Source tracebacks embed absolute file paths in the BIR, which makes the
# NEFF compile cache path-dependent (a fresh checkout would recompile for
# ~76s) and slows compilation. Disable before the module is built.
os.environ.setdefault("BASS_DISABLE_FRAME_TO_TRACEBACK", "1")

import numpy as np
import ml_dtypes

import concourse.bass as bass
import concourse.bacc as bacc
import concourse.tile as tile
from concourse import mybir
from concourse.bass_utils import run_bass_kernel_spmd

f32 = mybir.dt.float32
bf16 = mybir.dt.bfloat16
AF = mybir.ActivationFunctionType
OP = mybir.AluOpType

N_CORES = 8
NS = 5  # shifts -2..2
OUT_SCALE = 4.0 / 127.0  # int8 output quant step (BN output is ~N(0,1))
F1_SCALE = 4.0 / 127.0   # int8 quant step for the f1 upload
F3_SCALE = 4.0 / 127.0   # int8 quant step for the f3 upload
EPS = 1e-5
BN_N = 4 * 128 * 128  # elements per channel for batch stats


def bcast(ap, n, dim):
    """Insert a broadcast (step-0) dim of size n at position dim."""
    new = [list(p) for p in ap.ap]
    new.insert(dim, [0, n])
    return bass.AP(tensor=ap.tensor, offset=ap.offset, ap=new)


def build_module():
    nc = bacc.Bacc("TRN2", target_bir_lowering=False, debug=False,
                   num_devices=N_CORES)
    # ONE merged per-call upload (a second device_put costs ~50ms of
    # non-pipelined fixed overhead on the tunnel):
    #   cols 0:8192      = f1 INTERIOR rows y0..y0+63 as [64,128] int8
    #                      (the 3-row halos are exchanged between the
    #                      row-half pair cores via a masked AllReduce)
    #   cols 8192:12288  = omf (f3 offset-conv half): channel q's 8192
    #                      bytes as rows 0:32 on partition 2q and rows
    #                      32:64 on partition 2q+1; partitions 54:64 pad
    dat_d = nc.dram_tensor("dat", [64, 12288], mybir.dt.int8,
                           kind="ExternalInput")
    wp_d = nc.dram_tensor("wpack", [128, 659], bf16, kind="ExternalInput")
    # spack col 4 = mask_even (1 on even cores), col 5 = mask_odd
    sp_d = nc.dram_tensor("spack", [128, 6], f32, kind="ExternalInput")
    out_d = nc.dram_tensor("out", [64, 64, 128], mybir.dt.int8,
                           kind="ExternalOutput")

    import itertools
    cp_engines = itertools.cycle([0, 1])

    def cp(out, in_):
        if next(cp_engines) == 0:
            nc.vector.tensor_copy(out, in_)
        else:
            nc.scalar.copy(out, in_)

    with tile.TileContext(nc) as tc:
        import contextlib
        ctx = contextlib.ExitStack()
        with ctx:
            const = ctx.enter_context(tc.tile_pool(name="const", bufs=1))
            fld = ctx.enter_context(tc.tile_pool(name="fld", bufs=1))
            dram = ctx.enter_context(tc.tile_pool(name="dram", bufs=1,
                                                  space="DRAM"))
            tmpp = ctx.enter_context(tc.tile_pool(name="tmpp", bufs=1))
            outp = ctx.enter_context(tc.tile_pool(name="outp", bufs=1))
            bfp = ctx.enter_context(tc.tile_pool(name="bfp", bufs=1))

            # ---- constants in ----
            ow_sb = const.tile([64, 9, 27], bf16)
            nc.sync.dma_start(out=ow_sb, in_=wp_d[0:64, 0:243])
            wk_sb = const.tile([64, 576], bf16)
            nc.sync.dma_start(out=wk_sb[:, 0:288], in_=wp_d[0:64, 243:531])
            nc.sync.dma_start(out=wk_sb[:, 288:576], in_=wp_d[64:128, 243:531])
            ident = const.tile([128, 128], bf16)
            nc.sync.dma_start(out=ident, in_=wp_d[:, 531:659])
            sp_sb = const.tile([128, 6], f32)
            nc.sync.dma_start(out=sp_sb, in_=sp_d[:])
            gb_sb = const.tile([2, 2, 32], f32)
            nc.sync.dma_start(out=gb_sb, in_=sp_d[:, 3:4])
            syc = const.tile([128, NS, 64], bf16)
            sxc = const.tile([128, NS, 64], bf16)
            for i in range(NS):
                nc.vector.memset(syc[:, i, :], float(i - 2))
                nc.vector.memset(sxc[:, i, :], float(i - 2))

            # ---- phase 1: offset conv + fields ----
            phase1 = contextlib.ExitStack()
            xcp = phase1.enter_context(tc.tile_pool(name="xcp", bufs=1))
            fldA = phase1.enter_context(tc.tile_pool(name="fldA", bufs=1))
            omp = phase1.enter_context(tc.tile_pool(name="omp", bufs=2))
            pom = phase1.enter_context(tc.tile_pool(name="pom", bufs=2,
                                                    space="PSUM"))
            ptr = phase1.enter_context(tc.tile_pool(name="ptr", bufs=2,
                                                    space="PSUM"))

            # f1 interior int8; freed with this pool at phase1.close()
            dat_sb = xcp.tile([64, 64, 128], mybir.dt.int8)
            nc.sync.dma_start(
                out=dat_sb,
                in_=dat_d[:, 0:8192].rearrange("p (y x) -> p y x", y=64))
            # host-computed f3 half of the offset conv (+bias), int8 with
            # per-channel scales in spack col 2; un-permute the packed
            # [54 part, 4096B] DRAM layout to [27 part, 8192B] in one
            # affine DMA (src partition = 2q+h)
            omf_sb = xcp.tile([27, 64, 128], mybir.dt.int8)
            dst = omf_sb[:]
            dd = dat_d[:]
            nc.sync.dma_start(
                out=bass.AP(tensor=dst.tensor, offset=dst.offset,
                            ap=[list(dst.ap[0]), [4096, 2], [1, 4096]]),
                in_=bass.AP(tensor=dd.tensor, offset=dd.offset + 8192,
                            ap=[[2 * 12288, 27], [12288, 2], [1, 4096]]))

            # ---- halo exchange between pair cores (2b, 2b+1) ----
            # slot0 = even core's interior rows 61..63 (odd's above halo),
            # slot1 = odd core's interior rows 0..2 (even's below halo);
            # role masks make the SPMD program branchless, and re-masking
            # at read time yields exact zeros at the image boundary.
            hx1 = omp.tile([64, 2, 3, 128], bf16, tag="hx1")
            nc.vector.tensor_copy(hx1[:, 0], dat_sb[:, 61:64, :])
            nc.vector.tensor_copy(hx1[:, 1], dat_sb[:, 0:3, :])
            hxm = omp.tile([64, 2, 3, 128], f32, tag="hxm")
            nc.vector.tensor_scalar(hxm[:, 0], hx1[:, 0],
                                    sp_sb[0:64, 4:5], None, OP.mult)
            nc.vector.tensor_scalar(hxm[:, 1], hx1[:, 1],
                                    sp_sb[0:64, 5:6], None, OP.mult)
            cch_in = dram.tile([64, 2, 3, 128], f32)
            cch_out = dram.tile([64, 2, 3, 128], f32)
            nc.sync.dma_start(out=cch_in[:], in_=hxm)
            nc.gpsimd.collective_compute(
                "AllReduce", OP.add,
                replica_groups=[[2 * b, 2 * b + 1] for b in range(4)],
                ins=[cch_in[:]], outs=[cch_out[:]])
            halo32 = fld.tile([64, 2, 3, 128], f32, tag="halo")
            nc.sync.dma_start(out=halo32, in_=cch_out[:])

            xcat = xcp.tile([64, 66, 130], bf16)
            nc.vector.memset(xcat, 0.0)
            # int8 -> bf16 integer conversion (quant step folded into ow);
            # rows: 0 = above-halo row y0-1, 1:65 = interior, 65 = y0+64
            nc.vector.tensor_copy(xcat[:, 1:65, 1:129], dat_sb)
            nc.vector.tensor_scalar(xcat[:, 0:1, 1:129], halo32[:, 0, 2:3, :],
                                    sp_sb[0:64, 5:6], None, OP.mult)
            nc.vector.tensor_scalar(xcat[:, 65:66, 1:129],
                                    halo32[:, 1, 0:1, :],
                                    sp_sb[0:64, 4:5], None, OP.mult)

            om_T = fldA.tile([128, 64, 27], bf16, tag="omT")
            for c in range(16):  # chunks of 4 output rows
                ps = pom.tile([27, 512], f32)
                for k in range(9):
                    ky, kx = k // 3, k % 3
                    rhs = xcat[:, 4 * c + ky:4 * c + ky + 4, kx:kx + 128]
                    nc.tensor.matmul(ps, ow_sb[:, k, :], rhs,
                                     start=(k == 0), stop=(k == 8))
                omf_ch = omp.tile([27, 4, 128], bf16, tag="omfch")
                # int8 -> bf16 integers (proven DVE copy), then scale by
                # the per-channel quant step from spack col 2
                nc.vector.tensor_copy(omf_ch, omf_sb[:, 4 * c:4 * c + 4, :])
                nc.vector.tensor_scalar(omf_ch, omf_ch, sp_sb[0:27, 2:3],
                                        None, OP.mult)
                om_ch = omp.tile([27, 4, 128], bf16)
                nc.vector.tensor_tensor(
                    out=om_ch, in0=ps.rearrange("p (y x) -> p y x", y=4),
                    in1=omf_ch, op=OP.add)
                pt = ptr.tile([128, 4, 28], bf16)
                for j in range(4):
                    nc.tensor.transpose(pt[:, j, 0:27], om_ch[:, j, :],
                                        ident[0:27, 0:27])
                cp(om_T[:, 4 * c:4 * c + 4, :], pt[:, :, 0:27])

            # fields: cym [x, 9, 5, 64] (mask folded), cx [x, 9, 5, 64]
            sg = fldA.tile([128, 9, 64], bf16, tag="sg")
            nc.scalar.activation(
                sg, om_T[:, :, 18:27].rearrange("x y k -> x k y"), AF.Sigmoid)
            cym = fldA.tile([128, 9, NS, 64], bf16, tag="cym")
            dy_ap = om_T[:, :, 0:18:2].rearrange("x y k -> x k y")
            nc.vector.tensor_tensor(out=cym, in0=bcast(dy_ap, NS, 2),
                                    in1=bcast(syc, 9, 1), op=OP.subtract)
            nc.scalar.activation(cym, cym, AF.Abs)
            nc.vector.tensor_scalar(cym, cym, -1.0, 1.0, OP.mult, OP.add)
            nc.vector.tensor_scalar(cym, cym, 0.0, None, OP.max)
            nc.vector.tensor_tensor(out=cym, in0=cym, in1=bcast(sg, NS, 2),
                                    op=OP.mult)
            cx = fldA.tile([128, 9, NS, 64], bf16, tag="cx")
            dx_ap = om_T[:, :, 1:18:2].rearrange("x y k -> x k y")
            nc.vector.tensor_tensor(out=cx, in0=bcast(dx_ap, NS, 2),
                                    in1=bcast(sxc, 9, 1), op=OP.subtract)
            nc.scalar.activation(cx, cx, AF.Abs)
            nc.vector.tensor_scalar(cx, cx, -1.0, 1.0, OP.mult, OP.add)
            nc.vector.tensor_scalar(cx, cx, 0.0, None, OP.max)
            # Bf[x, k, sx, sy, y] = cx * cym (coefficients at the OUTPUT pixel)
            Bf = bfp.tile([128, 9, NS, NS, 64], bf16)
            nc.vector.tensor_tensor(out=Bf, in0=bcast(cx, NS, 3),
                                    in1=bcast(cym, NS, 2), op=OP.mult)
            phase1.close()

            # ---- phase 2: g = per-tap 1x1 conv on padded rows ----
            stackA = contextlib.ExitStack()
            gp = stackA.enter_context(tc.tile_pool(name="gp", bufs=1))
            featp = contextlib.ExitStack()
            fpool = featp.enter_context(tc.tile_pool(name="fpool", bufs=1))
            pg = featp.enter_context(tc.tile_pool(name="pg", bufs=2,
                                                  space="PSUM"))
            feat_i8 = fpool.tile([64, 64, 128], mybir.dt.int8)
            nc.sync.dma_start(
                out=feat_i8,
                in_=dat_d[:, 0:8192].rearrange("p (y x) -> p y x", y=64))
            feat_sb = fpool.tile([64, 70, 128], bf16)
            # int8 -> bf16 integers; f1 quant step folded into wk;
            # rows 0:3 / 67:70 come from the exchanged halos (masked to
            # exact zeros at the image boundary)
            nc.vector.tensor_copy(feat_sb[:, 3:67, :], feat_i8)
            nc.vector.tensor_scalar(feat_sb[:, 0:3, :], halo32[:, 0, :, :],
                                    sp_sb[0:64, 5:6], None, OP.mult)
            nc.vector.tensor_scalar(feat_sb[:, 67:70, :], halo32[:, 1, :, :],
                                    sp_sb[0:64, 4:5], None, OP.mult)

            g = gp.tile([128, 9, 64, 70], bf16)
            ga = g[:]

            def gdst(r):
                # [(2 chunks), 288 (k,o)-cols] view of g[:, :, :, r]
                return bass.AP(tensor=ga.tensor, offset=ga.offset + r,
                               ap=[list(ga.ap[0]), [20160, 2], [70, 288]])

            for r in range(70):
                psg = pg.tile([128, 2, 512], f32)
                nc.tensor.matmul(psg[:, 0, 0:288], feat_sb[:, r, :],
                                 wk_sb[:, 0:288], start=True, stop=True)
                nc.tensor.matmul(psg[:, 1, 0:288], feat_sb[:, r, :],
                                 wk_sb[:, 288:576], start=True, stop=True)
                cp(gdst(r), psg[:, 0:2, 0:288])
            featp.close()

            # ---- phase 3: flat warp sum over shifted-g planes ----
            # acc[x, o, y] = sum_{k,sx,sy} Bf[x,k,sx,sy,y]
            #                  * g[x+dlt, k, o, y+ky+sy],  dlt = kx-1+sx.
            # x-shifts of g via partition-shifted SBUF DMAs (per dlt, kx
            # plane group); 5 sy taps fused per op via a sliding-window AP
            # then reduced; o processed in halves to bound tmp size.
            gsp = stackA.enter_context(tc.tile_pool(name="gsp", bufs=1))
            acc = outp.tile([128, 64, 64], bf16)
            Gs = gsp.tile([128, 3, 64, 70], bf16)

            def ywin(ap, off):
                # [.., n(stride 1)] -> [.., 64, 5] sliding window at +off
                new = [list(p) for p in ap.ap[:-1]] + [[1, 64], [1, 5]]
                return bass.AP(tensor=ap.tensor, offset=ap.offset + off,
                               ap=new)

            first = [True]

            def warp_terms(slc, delta, kx):
                # slc(ky): [128, 64(o), 70(y')] plane for this kx
                sxi = delta - kx + 3
                for ky in range(3):
                    k = 3 * ky + kx
                    gw = ywin(slc(ky), ky)
                    bf_ap = bcast(Bf[:, k, sxi, :, :], 64, 1).rearrange(
                        "x o s y -> x o y s")
                    tmp = tmpp.tile([128, 64, 64, NS], bf16, tag="t")
                    nc.vector.tensor_tensor(out=tmp, in0=gw, in1=bf_ap,
                                            op=OP.mult)
                    tmp2 = tmpp.tile([128, 64, 64], f32, tag="t2")
                    nc.vector.tensor_reduce(tmp2, tmp,
                                            axis=mybir.AxisListType.X,
                                            op=OP.add)
                    if first[0]:
                        nc.vector.tensor_copy(acc, tmp2)
                        first[0] = False
                    else:
                        nc.vector.tensor_tensor(out=acc, in0=acc, in1=tmp2,
                                                op=OP.add)

            for kx in range(3):
                warp_terms(
                    lambda ky, kx=kx: g[:, 3 * ky + kx, :, :], 0, kx)
            for delta in (-3, -2, -1, 1, 2, 3):
                # quadrant-aligned memset band once per delta; the shift
                # DMAs only ever write the interior, so the edge stays zero
                # across the kx iterations.
                if delta > 0:
                    nc.vector.memset(Gs[96:128, :, :, :], 0.0)
                else:
                    nc.vector.memset(Gs[0:32, :, :, :], 0.0)
                for kx in range(max(0, delta - 1), min(2, delta + 3) + 1):
                    if delta > 0:
                        nc.sync.dma_start(
                            out=Gs[0:128 - delta, :, :, :],
                            in_=g[delta:128, kx:9:3, :, :])
                    else:
                        d = -delta
                        nc.sync.dma_start(
                            out=Gs[d:128, :, :, :],
                            in_=g[0:128 - d, kx:9:3, :, :])
                    warp_terms(lambda ky: Gs[:, ky, :, :], delta, kx)

            stackA.close()  # free g + Gs

            # ---- transpose acc -> hacc [(par,y), j, x] ----
            hp = ctx.enter_context(tc.tile_pool(name="hp", bufs=1))
            pv = ctx.enter_context(tc.tile_pool(name="pv", bufs=2,
                                                space="PSUM"))
            pst = ctx.enter_context(tc.tile_pool(name="pst", bufs=1,
                                                 space="PSUM"))
            hacc = hp.tile([128, 32, 128], bf16)
            for j2 in range(4):
                pvt = pv.tile([128, 8, 128], bf16)
                for jj in range(8):
                    j = 8 * j2 + jj
                    nc.tensor.transpose(
                        pvt[:, jj, :],
                        acc[:, 2 * j:2 * j + 2, :].rearrange(
                            "x o y -> x (o y)"),
                        ident)
                cp(hacc[:, 8 * j2:8 * j2 + 8, :], pvt)

            # ---- BN stats ----
            sq = hp.tile([128, 32, 128], bf16, tag="sq")
            nc.vector.tensor_tensor(out=sq, in0=hacc, in1=hacc, op=OP.mult)
            stat2 = fld.tile([128, 2, 32], f32, tag="st2")
            nc.vector.tensor_reduce(stat2[:, 0, :], hacc,
                                    axis=mybir.AxisListType.X, op=OP.add)
            nc.vector.tensor_reduce(stat2[:, 1, :], sq,
                                    axis=mybir.AxisListType.X, op=OP.add)
            ps1 = pst.tile([2, 2, 32], f32)
            nc.tensor.matmul(ps1.rearrange("p a b -> p (a b)"), sp_sb[:, 0:2],
                             stat2.rearrange("p a b -> p (a b)"),
                             start=True, stop=True)
            st_sb = fld.tile([2, 2, 32], f32, tag="stsb")
            nc.vector.tensor_copy(st_sb, ps1)
            cc_in = dram.tile([2, 2, 32], f32)
            cc_out = dram.tile([2, 2, 32], f32)
            nc.sync.dma_start(out=cc_in[:], in_=st_sb)
            nc.gpsimd.collective_compute(
                "AllReduce", OP.add,
                replica_groups=[list(range(N_CORES))],
                ins=[cc_in[:]], outs=[cc_out[:]])
            red = fld.tile([2, 2, 32], f32, tag="red")
            nc.sync.dma_start(out=red, in_=cc_out[:])

            mt = fld.tile([2, 32], f32, tag="mt")
            nc.vector.tensor_scalar(mt, red[:, 0, :], 1.0 / BN_N, None,
                                    OP.mult)
            ex2 = fld.tile([2, 32], f32, tag="ex2")
            nc.vector.tensor_scalar(ex2, red[:, 1, :], 1.0 / BN_N, None,
                                    OP.mult)
            var = fld.tile([2, 32], f32, tag="var")
            nc.vector.tensor_tensor(out=var, in0=mt, in1=mt, op=OP.mult)
            nc.vector.tensor_tensor(out=var, in0=ex2, in1=var, op=OP.subtract)
            nc.vector.tensor_scalar(var, var, EPS, None, OP.add)
            sqv = fld.tile([2, 32], f32, tag="sqv")
            nc.scalar.activation(sqv, var, AF.Sqrt)
            rstd = fld.tile([2, 32], f32, tag="rstd")
            nc.vector.reciprocal(rstd, sqv)
            AB = fld.tile([2, 2, 32], f32, tag="AB")
            nc.vector.tensor_tensor(out=AB[:, 0, :], in0=gb_sb[:, 0, :],
                                    in1=rstd, op=OP.mult)
            nc.vector.tensor_tensor(out=AB[:, 1, :], in0=mt, in1=AB[:, 0, :],
                                    op=OP.mult)
            nc.vector.tensor_tensor(out=AB[:, 1, :], in0=gb_sb[:, 1, :],
                                    in1=AB[:, 1, :], op=OP.subtract)
            ab_d = dram.tile([2, 2, 32], f32)
            nc.sync.dma_start(out=ab_d[:], in_=AB)
            ABc = fld.tile([128, 2, 32], f32, tag="ABc")
            nc.sync.dma_start(
                out=ABc,
                in_=bass.AP(tensor=ab_d.tensor, offset=ab_d.offset,
                            ap=[[64, 2], [0, 64], [32, 2], [1, 32]]))

            # ---- BN apply + int8 quantize + store (one DMA out) ----
            # gamma/beta are pre-divided by OUT_SCALE on the host, so
            # fin = hacc*A + B is already in quant units; clamp to the
            # int8 range (avoids wraparound on the ~1e-5 tail), convert.
            fin = hp.tile([128, 32, 128], f32)
            nc.vector.tensor_tensor(out=fin, in0=hacc,
                                    in1=bcast(ABc[:, 0, :], 128, 2),
                                    op=OP.mult)
            nc.vector.tensor_tensor(out=fin, in0=fin,
                                    in1=bcast(ABc[:, 1, :], 128, 2),
                                    op=OP.add)
            nc.vector.tensor_scalar(fin, fin, 127.0, None, OP.min)
            nc.vector.tensor_scalar(fin, fin, -127.0, None, OP.max)
            finq = hp.tile([128, 32, 128], mybir.dt.int8, tag="finq")
            nc.vector.tensor_copy(finq, fin)
            od = out_d[:]
            out_ap = bass.AP(tensor=od.tensor, offset=od.offset,
                             ap=[[8192, 2], [128, 64], [16384, 32], [1, 128]])
            nc.sync.dma_start(out=out_ap, in_=finq)

    nc.finalize()

    # The BIR debug_table embeds absolute source paths, which would make
    # the NEFF compile cache path-dependent (a fresh checkout would pay a
    # ~76s recompile). Normalize them in the serialized module.
    import re
    orig_to_json_bytes = nc.to_json_bytes

    def to_json_bytes_scrubbed():
        return re.sub(rb'"filename":"[^"]*"', b'"filename":"kernel.py"',
                      orig_to_json_bytes())

    nc.to_json_bytes = to_json_bytes_scrubbed
    return nc


_module_cache = {}


def get_module():
    if "m" not in _module_cache:
        _module_cache["m"] = build_module()
    return _module_cache["m"]


def _quant_i8(x, scale):
    # round(x/scale) clipped to [-127,127] in 4 passes:
    # t = clip(x/s, ...) + 127.5; floor via uint8 cast; -127 with wraparound
    # (uint8 mod-256 arithmetic == int8 two's complement).
    t = np.multiply(x, 1.0 / scale, dtype=np.float32)
    np.clip(t, -127.0, 127.0, out=t)
    t += 127.5
    q = t.astype(np.uint8)
    q -= 127
    return q.view(np.int8)


def _om_f3_host(f3, ow_f3, ob):
    """conv3x3(f3, ow[:, 64:]) + ob -> [4, 27, 128, 128] f32, via 36
    row-blocked skinny GEMMs on the zero-padded image."""
    B = f3.shape[0]
    Xp = np.zeros((B, 64, 130, 130), np.float32)
    Xp[:, :, 1:129, 1:129] = f3
    om = np.broadcast_to(ob[None, :, None, None], (B, 27, 128, 128)).copy()
    for ky in range(3):
        for kx in range(3):
            Wk = np.ascontiguousarray(ow_f3[:, :, ky, kx])  # [27, 64]
            for b in range(B):
                blk = Xp[b, :, ky:ky + 128, :].reshape(64, 128 * 130)
                t = (Wk @ blk).reshape(27, 128, 130)
                om[b] += t[:, :, kx:kx + 128]
    return om


def prep_global(f1_feat, f3_feat, offset_w, offset_b, main_w, gamma, beta):
    """Host-side packing into the CONCATENATED (8*shape0) global arrays
    that the sharded runner consumes directly."""
    bf = ml_dtypes.bfloat16
    f1q = _quant_i8(np.asarray(f1_feat, np.float32), F1_SCALE)  # [4,64,128,128]
    ow = np.asarray(offset_w, np.float32)   # [27,128,3,3]
    ob = np.asarray(offset_b, np.float32)
    wk = np.asarray(main_w, np.float32)     # [64,64,3,3]

    # merged per-core upload [64, 12288] int8: f1 INTERIOR rows
    # y0..y0+63 in cols 0:8192 (halos exchanged on device), the packed
    # omf in cols 8192:12288 (see build_module).
    dat = np.empty((N_CORES, 64, 12288), np.int8)
    dv = dat[:, :, 0:8192].reshape(N_CORES, 64, 64, 128)
    for i in range(N_CORES):
        b, half = i // 2, i % 2
        dv[i] = f1q[b][:, 64 * half:64 * half + 64, :]

    # f3 enters only through the 27-channel offset conv; computing that
    # contribution host-side shrinks its upload 64ch -> 27ch. Quantize
    # per-channel (scales land in spack col 2).
    omf = _om_f3_host(np.asarray(f3_feat, np.float32), ow[:, 64:128], ob)
    sc = np.maximum(np.abs(omf).max(axis=(0, 2, 3)) / 127.0, 1e-30)  # [27]
    omq = np.clip(np.rint(omf / sc[None, :, None, None]), -127, 127) \
        .astype(np.int8)
    for i in range(N_CORES):
        b, half = i // 2, i % 2
        # [27,64,128] -> [27,2,4096] -> partitions 2q+h
        dat[i, 0:54, 8192:12288] = \
            omq[b][:, 64 * half:64 * half + 64, :].reshape(54, 4096)
        dat[i, 54:64, 8192:12288] = 0

    # wpack: ow_t f1-half [64,243] (rows 64:128 unused) | wk packed
    # [128,288] | ident [128,128]; int8 quant steps folded in
    ow_t = np.zeros((128, 243), np.float32)
    ow_t[0:64] = ow[:, 0:64].reshape(27, 64, 9).transpose(1, 2, 0) \
        .reshape(64, 243) * F1_SCALE
    wk_t = wk.reshape(64, 64, 9).transpose(1, 2, 0).reshape(64, 576) * F1_SCALE
    wk_r = np.concatenate([wk_t[:, 0:288], wk_t[:, 288:576]], axis=0)
    wpack = np.concatenate(
        [ow_t, wk_r, np.eye(128, dtype=np.float32)], axis=1).astype(bf)

    # spack: sel cols 0-1 | omf per-channel scales col 2 | gb flat col 3
    # | halo role masks cols 4-5 (per-core: even, odd)
    spack = np.zeros((128, 6), np.float32)
    spack[0:64, 0] = 1.0
    spack[64:128, 1] = 1.0
    spack[0:27, 2] = sc
    # pre-divide gamma/beta by OUT_SCALE so the on-device BN affine lands
    # directly in int8 quant units
    gam = np.asarray(gamma, np.float32) / OUT_SCALE
    bet = np.asarray(beta, np.float32) / OUT_SCALE
    gb = np.zeros((2, 2, 32), np.float32)
    for par in range(2):
        gb[par, 0, :] = gam[par::2]
        gb[par, 1, :] = bet[par::2]
    spack[:, 3] = gb.reshape(-1)
    # per-core role masks for the halo exchange
    spack_all = np.tile(spack, (N_CORES, 1)).reshape(N_CORES, 128, 6)
    for i in range(N_CORES):
        spack_all[i, :, 4] = 1.0 if i % 2 == 0 else 0.0
        spack_all[i, :, 5] = 0.0 if i % 2 == 0 else 1.0

    # wpack is per-core [128, ...] (tiled to the global layout only on
    # rare weight re-upload); spack carries per-core masks so it ships
    # as the full (8*128, 6) array.
    return (dat.reshape(N_CORES * 64, 12288), wpack,
            spack_all.reshape(N_CORES * 128, 6))


class _AxonRunner:
    """Persistent PJRT runner: one shard_map jit, resident weights,
    donated output scratch chained from the previous call."""

    def __init__(self, nc):
        import jax
        import warnings
        from jax.sharding import Mesh, PartitionSpec, NamedSharding
        with warnings.catch_warnings():
            warnings.simplefilter("ignore")
            from jax.experimental.shard_map import shard_map
        from concourse.bass2jax import (
            _bass_exec_p, install_neuronx_cc_hook, partition_id_tensor)

        install_neuronx_cc_hook()
        self.jax = jax
        self.nc = nc

        partition_name = (nc.partition_id_tensor.name
                          if nc.partition_id_tensor else None)
        in_names, out_names, out_avals = [], [], []
        for alloc in nc.m.functions[0].allocations:
            if not isinstance(alloc, mybir.MemoryLocationSet):
                continue
            name = alloc.memorylocations[0].name
            if alloc.kind == "ExternalInput":
                if name != partition_name:
                    in_names.append(name)
            elif alloc.kind == "ExternalOutput":
                out_names.append(name)
                out_avals.append(jax.core.ShapedArray(
                    tuple(alloc.tensor_shape), mybir.dt.np(alloc.dtype)))
        assert in_names == ["dat", "wpack", "spack"], in_names
        assert out_names == ["out"], out_names
        n_params = len(in_names)
        n_outs = len(out_avals)
        all_names = list(in_names) + list(out_names)
        if partition_name is not None:
            all_names.append(partition_name)

        def _body(*args):
            operands = list(args)
            if partition_name is not None:
                operands.append(partition_id_tensor())
            outs = _bass_exec_p.bind(
                *operands, out_avals=tuple(out_avals),
                in_names=tuple(all_names), out_names=tuple(out_names),
                lowering_input_output_aliases=(),
                sim_require_finite=True, sim_require_nnan=True, nc=nc)
            return tuple(outs)

        devices = jax.devices()[:N_CORES]
        mesh = Mesh(np.asarray(devices), ("core",))
        self.sh = NamedSharding(mesh, PartitionSpec("core"))
        self.jfn = jax.jit(
            shard_map(_body, mesh=mesh,
                      in_specs=(PartitionSpec("core"),) * (n_params + n_outs),
                      out_specs=(PartitionSpec("core"),) * n_outs,
                      check_rep=False),
            donate_argnums=tuple(range(n_params, n_params + n_outs)),
            keep_unused=True)

        self.w_key = None
        self.w_dev = None   # (wpack_dev, spack_dev)
        self.scratch = None

    def __call__(self, dat_all, wpack, spack_all):
        jax = self.jax
        # weights: resident unless their bytes change (1.4MB -> the
        # hash check is ~0.5ms); spack_all is already the full global
        key = (wpack.tobytes(), spack_all.tobytes())
        if self.w_key != key:
            self.w_dev = (jax.device_put(np.tile(wpack, (N_CORES, 1)),
                                         self.sh),
                          jax.device_put(spack_all, self.sh))
            self.w_key = key
        if self.scratch is None:
            self.scratch = jax.device_put(
                np.zeros((N_CORES * 64, 64, 128), np.int8), self.sh)
        dat_dev = jax.device_put(dat_all, self.sh)
        outs = self.jfn(dat_dev, self.w_dev[0], self.w_dev[1], self.scratch)
        out = outs[0]
        # the kernel writes every element of out, so the previous output
        # buffer is a valid scratch donation for the next call (the caller
        # materializes host copies before the next call happens)
        self.scratch = out
        return out  # global jax array [8*64, 64, 128] int8


def _get_runner():
    if "r" not in _module_cache:
        _module_cache["r"] = _AxonRunner(get_module())
    return _module_cache["r"]


def run_device(dat_all, wpack, spack):
    """One full device round: upload dat, run 8 cores, download +
    unpack to the final [4, 64, 128, 128] f32 output."""
    from concourse._compat import axon_active
    out = np.empty((4, 64, 128, 128), np.float32)
    s = np.float32(OUT_SCALE)
    if axon_active():
        res = _get_runner()(dat_all, wpack, spack)  # [512,64,128] i8
        # fetch per-shard and dequantize each as it lands, so the host
        # unpack overlaps the tail of the tunnel download
        by_row = {}
        for sd in res.addressable_shards:
            sd.data.copy_to_host_async()
            by_row[sd.index[0].start or 0] = sd.data
        for i in range(N_CORES):
            b, h = i // 2, i % 2
            np.multiply(np.asarray(by_row[64 * i]), s,
                        out=out[b, :, 64 * h:64 * h + 64, :],
                        casting="unsafe")
        return out
    nc = get_module()
    maps = [{"dat": dat_all[64 * i:64 * i + 64], "wpack": wpack,
             "spack": spack[128 * i:128 * i + 128]}
            for i in range(N_CORES)]
    rr = run_bass_kernel_spmd(nc, maps, core_ids=list(range(N_CORES)))
    for i in range(N_CORES):
        b, h = i // 2, i % 2
        np.multiply(rr.results[i]["out"], s,
                    out=out[b, :, 64 * h:64 * h + 64, :], casting="unsafe")
    return out


def _bn_ok(out, gamma, beta):
    """BN output invariant: per-channel mean==beta, std≈|gamma| (batch
    statistics are computed from this very tensor). Good runs deviate
    <5e-4; a corrupted round (rare tunnel/device glitch) trips this."""
    g = np.abs(np.asarray(gamma, np.float32))
    b = np.asarray(beta, np.float32)
    ref = np.maximum(g, 1e-3)
    m = out.mean(axis=(0, 2, 3))
    s = out.std(axis=(0, 2, 3))
    return bool((np.abs(m - b) <= 0.02 * ref).all()
                and (np.abs(s - g) <= 0.03 * ref).all())


def _kernel_subprocess(**inputs):
    """Rerun in a fresh process. The device sporadically hard-crashes
    (NRT_EXEC_UNIT_UNRECOVERABLE, ~1/300 rounds) which poisons the PJRT
    client for the whole process; a fresh process recovers (NEFF compile
    is disk-cached, so this costs ~3s)."""
    import subprocess
    import sys
    import tempfile
    me = os.path.abspath(__file__)
    with tempfile.TemporaryDirectory() as td:
        np.savez(os.path.join(td, "in.npz"),
                 **{k: np.asarray(v) for k, v in inputs.items()})
        code = (
            "import os, numpy as np, importlib.util\n"
            "os.environ['KERNEL_NO_SUBPROC'] = '1'\n"
            f"spec = importlib.util.spec_from_file_location('kmod', {me!r})\n"
            "m = importlib.util.module_from_spec(spec)\n"
            "spec.loader.exec_module(m)\n"
            f"d = np.load(os.path.join({td!r}, 'in.npz'))\n"
            "out = m.kernel(**{k: d[k] for k in d.files})\n"
            f"np.save(os.path.join({td!r}, 'out.npy'), out)\n")
        subprocess.run([sys.executable, "-c", code], check=True)
        return np.load(os.path.join(td, "out.npy"))


def kernel(**inputs):
    packed = prep_global(**inputs)
    try:
        out = run_device(*packed)
        for _ in range(2):
            if _bn_ok(out, inputs["gamma"], inputs["beta"]):
                break
            r = _module_cache.get("r")
            if r is not None:
                r.w_key = None  # force weight re-upload on the retry
            out = run_device(*packed)
        return out
    except Exception:
        if os.environ.get("KERNEL_NO_SUBPROC"):
            raise
        return _kernel_subprocess(**inputs)


if __name__ == "__main__":
    d = np.load("/root/problem/ref_cache.npz")
    inp = {k: d[k] for k in d.files if k != "expected"}
    got = kernel(**inp)
    exp = d["expected"]
    err = np.linalg.norm(got - exp) / np.linalg.norm(exp)
    print("rel l2 err:", err, "maxabs:", np.abs(got - exp).max())


# revision 56
# speedup vs baseline: 1.0287x; 1.0287x over previous
"""Trainium2 Bass kernel for nn_DeformableAlignment.

Sharding: 8 cores = (batch b in 0..4) x (image row-half in {0,1}).
Each core computes out[b, :, y0:y0+64, :] for y0 = 64*(i%2).

Math (per core, matches reference exactly):
  om  = conv3x3(f1-half on device) + omf (f3-half, host) [27, 64, 128]
  dy/dx per tap k; sg = sigmoid(mask-channels)
  bilinear warp written floor-free via hat fields at the OUTPUT pixel:
    Bf[k,sx,sy] = relu(1-|dx-sx|) * relu(1-|dy-sy|)*sg  (sx,sy in -2..2)
  g[k] = 1x1-conv of f1 with main_w tap k, computed ONCE on the
         y-padded grid: g[x=128 part, 9k, 64o, 70y]     (140 matmuls)
  acc[x,o,y] = sum_{k,sx,sy} Bf[x,k,sx,sy,y] * g[x+dlt, k, o, y+ky+sy],
         dlt = kx-1+sx: x-shifts of g are 12 partition-shifted
         SBUF->SBUF DMAs (per dlt & kx plane); the 5 sy taps are fused
         per op via a sliding-window AP + reduce.
  BN stats via on-device partial sums + AllReduce across 8 cores; the
  BN affine is pre-divided by OUT_SCALE so the output quantizes to int8.

IO is minimized for the axon tunnel (aggregate ~45-70MB/s shared by
both directions, ~47ms dispatch RTT that pipelines away, and ~50ms of
NON-pipelined fixed cost per device_put -> everything rides ONE tensor):
  dat   [64, 12288] int8, merged per-call upload:
        cols 0:8192     f1 INTERIOR rows y0..y0+63 as [64,128],
                        quantized at 4/127 (clip 4sig; scale folded
                        into ow/wk host-side, so ints convert straight
                        to bf16). The 3-row warp halos are NOT
                        uploaded: the row-half pair cores (2b, 2b+1)
                        exchange them via a masked pair AllReduce
                        (role masks in spack cols 4-5 keep the SPMD
                        program branchless; re-masking at read time
                        yields exact zeros at the image boundary).
        cols 8192:12288 omf = the f3 half of the offset conv (+bias),
                        computed host-side (f3 only feeds this
                        27-channel conv, so shipping the contracted
                        result cuts that upload 64ch -> 27ch); channel
                        q's 8192B split across partitions 2q/2q+1,
                        un-permuted on device by one affine DMA;
                        per-channel scales ride in spack col 2.
  wpack [128, 659] bf16: ow_t f1-half [64,243] | wk packed [128,288]
        | ident (ow/wk scaled by the f1 quant step)
  spack [128, 6] f32 per core: sel | sel | omf scales |
        (gamma,beta)/OUT_SCALE | mask_even | mask_odd
  out   [64, 64, 128] int8 (dequantized to f32 on host; BN output is
        ~N(0,1) per channel so a 4/127 step keeps rel err ~1.75%)

Runner: under axon, run_bass_kernel_spmd would rebuild a jax.jit
(re-trace + re-lower, embedding the multi-MB BIR) and re-upload
weights + donated zero output buffers on EVERY call.  _AxonRunner
instead keeps one persistent shard_map jit, keeps wpack/spack resident
on device (re-uploaded only if the weight bytes change), and donates
the PREVIOUS call's output buffer as the next call's scratch (the
kernel writes every output element, so the scratch contents are
irrelevant after the first call's zeros).  Per call the tunnel moves
one 6.3MB upload and one 4.2MB download.  kernel() self-checks the BN
output invariant (per-channel mean==beta, std==|gamma|) and retries
the rare corrupted round; on the sporadic hard device crash
(NRT_EXEC_UNIT_UNRECOVERABLE poisons the process's PJRT client) it
recomputes in a fresh subprocess.
"""

import os

# Source tracebacks embed absolute file paths in the BIR, which makes the
# NEFF compile cache path-dependent (a fresh checkout would recompile for
# ~76s) and slows compilation. Disable before the module is built.
os.environ.setdefault("BASS_DISABLE_FRAME_TO_TRACEBACK", "1")

import numpy as np
import ml_dtypes

import concourse.bass as bass
import concourse.bacc as bacc
import concourse.tile as tile
from concourse import mybir
from concourse.bass_utils import run_bass_kernel_spmd

f32 = mybir.dt.float32
bf16 = mybir.dt.bfloat16
AF = mybir.ActivationFunctionType
OP = mybir.AluOpType

N_CORES = 8
NS = 5  # shifts -2..2
OUT_SCALE = 4.0 / 127.0  # int8 output quant step (BN output is ~N(0,1))
F1_SCALE = 4.0 / 127.0   # int8 quant step for the f1 upload
F3_SCALE = 4.0 / 127.0   # int8 quant step for the f3 upload
EPS = 1e-5
BN_N = 4 * 128 * 128  # elements per channel for batch stats


def bcast(ap, n, dim):
    """Insert a broadcast (step-0) dim of size n at position dim."""
    new = [list(p) for p in ap.ap]
    new.insert(dim, [0, n])
    return bass.AP(tensor=ap.tensor, offset=ap.offset, ap=new)


def build_module():
    nc = bacc.Bacc("TRN2", target_bir_lowering=False, debug=False,
                   num_devices=N_CORES)
    # ONE merged per-call upload (a second device_put costs ~50ms of
    # non-pipelined fixed overhead on the tunnel):
    #   cols 0:8192      = f1 INTERIOR rows y0..y0+63 as [64,128] int8
    #                      (the 3-row halos are exchanged between the
    #                      row-half pair cores via a masked AllReduce)
    #   cols 8192:12288  = omf (f3 offset-conv half): channel q's 8192
    #                      bytes as rows 0:32 on partition 2q and rows
    #                      32:64 on partition 2q+1; partitions 54:64 pad
    dat_d = nc.dram_tensor("dat", [64, 12288], mybir.dt.int8,
                           kind="ExternalInput")
    wp_d = nc.dram_tensor("wpack", [128, 659], bf16, kind="ExternalInput")
    # spack col 4 = mask_even (1 on even cores), col 5 = mask_odd
    sp_d = nc.dram_tensor("spack", [128, 6], f32, kind="ExternalInput")
    out_d = nc.dram_tensor("out", [64, 64, 128], mybir.dt.int8,
                           kind="ExternalOutput")

    import itertools
    cp_engines = itertools.cycle([0, 1])

    def cp(out, in_):
        if next(cp_engines) == 0:
            nc.vector.tensor_copy(out, in_)
        else:
            nc.scalar.copy(out, in_)

    with tile.TileContext(nc) as tc:
        import contextlib
        ctx = contextlib.ExitStack()
        with ctx:
            const = ctx.enter_context(tc.tile_pool(name="const", bufs=1))
            fld = ctx.enter_context(tc.tile_pool(name="fld", bufs=1))
            dram = ctx.enter_context(tc.tile_pool(name="dram", bufs=1,
                                                  space="DRAM"))
            tmpp = ctx.enter_context(tc.tile_pool(name="tmpp", bufs=1))
            outp = ctx.enter_context(tc.tile_pool(name="outp", bufs=1))
            bfp = ctx.enter_context(tc.tile_pool(name="bfp", bufs=1))

            # ---- constants in ----
            ow_sb = const.tile([64, 9, 27], bf16)
            nc.sync.dma_start(out=ow_sb, in_=wp_d[0:64, 0:243])
            wk_sb = const.tile([64, 576], bf16)
            nc.sync.dma_start(out=wk_sb[:, 0:288], in_=wp_d[0:64, 243:531])
            nc.sync.dma_start(out=wk_sb[:, 288:576], in_=wp_d[64:128, 243:531])
            ident = const.tile([128, 128], bf16)
            nc.sync.dma_start(out=ident, in_=wp_d[:, 531:659])
            sp_sb = const.tile([128, 6], f32)
            nc.sync.dma_start(out=sp_sb, in_=sp_d[:])
            gb_sb = const.tile([2, 2, 32], f32)
            nc.sync.dma_start(out=gb_sb, in_=sp_d[:, 3:4])
            syc = const.tile([128, NS, 64], bf16)
            sxc = const.tile([128, NS, 64], bf16)
            for i in range(NS):
                nc.vector.memset(syc[:, i, :], float(i - 2))
                nc.vector.memset(sxc[:, i, :], float(i - 2))

            # ---- phase 1: offset conv + fields ----
            phase1 = contextlib.ExitStack()
            xcp = phase1.enter_context(tc.tile_pool(name="xcp", bufs=1))
            fldA = phase1.enter_context(tc.tile_pool(name="fldA", bufs=1))
            omp = phase1.enter_context(tc.tile_pool(name="omp", bufs=2))
            pom = phase1.enter_context(tc.tile_pool(name="pom", bufs=2,
                                                    space="PSUM"))
            ptr = phase1.enter_context(tc.tile_pool(name="ptr", bufs=2,
                                                    space="PSUM"))

            # f1 interior int8; freed with this pool at phase1.close()
            dat_sb = xcp.tile([64, 64, 128], mybir.dt.int8)
            nc.sync.dma_start(
                out=dat_sb,
                in_=dat_d[:, 0:8192].rearrange("p (y x) -> p y x", y=64))
            # host-computed f3 half of the offset conv (+bias), int8 with
            # per-channel scales in spack col 2; un-permute the packed
            # [54 part, 4096B] DRAM layout to [27 part, 8192B] in one
            # affine DMA (src partition = 2q+h)
            omf_sb = xcp.tile([27, 64, 128], mybir.dt.int8)
            dst = omf_sb[:]
            dd = dat_d[:]
            nc.sync.dma_start(
                out=bass.AP(tensor=dst.tensor, offset=dst.offset,
                            ap=[list(dst.ap[0]), [4096, 2], [1, 4096]]),
                in_=bass.AP(tensor=dd.tensor, offset=dd.offset + 8192,
                            ap=[[2 * 12288, 27], [12288, 2], [1, 4096]]))

            # ---- halo exchange between pair cores (2b, 2b+1) ----
            # slot0 = even core's interior rows 61..63 (odd's above halo),
            # slot1 = odd core's interior rows 0..2 (even's below halo);
            # role masks make the SPMD program branchless, and re-masking
            # at read time yields exact zeros at the image boundary.
            hx1 = omp.tile([64, 2, 3, 128], bf16, tag="hx1")
            nc.vector.tensor_copy(hx1[:, 0], dat_sb[:, 61:64, :])
            nc.vector.tensor_copy(hx1[:, 1], dat_sb[:, 0:3, :])
            hxm = omp.tile([64, 2, 3, 128], f32, tag="hxm")
            nc.vector.tensor_scalar(hxm[:, 0], hx1[:, 0],
                                    sp_sb[0:64, 4:5], None, OP.mult)
            nc.vector.tensor_scalar(hxm[:, 1], hx1[:, 1],
                                    sp_sb[0:64, 5:6], None, OP.mult)
            cch_in = dram.tile([64, 2, 3, 128], f32)
            cch_out = dram.tile([64, 2, 3, 128], f32)
            nc.sync.dma_start(out=cch_in[:], in_=hxm)
            nc.gpsimd.collective_compute(
                "AllReduce", OP.add,
                replica_groups=[[2 * b, 2 * b + 1] for b in range(4)],
                ins=[cch_in[:]], outs=[cch_out[:]])
            halo32 = fld.tile([64, 2, 3, 128], f32, tag="halo")
            nc.sync.dma_start(out=halo32, in_=cch_out[:])

            xcat = xcp.tile([64, 66, 130], bf16)
            nc.vector.memset(xcat, 0.0)
            # int8 -> bf16 integer conversion (quant step folded into ow);
            # rows: 0 = above-halo row y0-1, 1:65 = interior, 65 = y0+64
            nc.vector.tensor_copy(xcat[:, 1:65, 1:129], dat_sb)
            nc.vector.tensor_scalar(xcat[:, 0:1, 1:129], halo32[:, 0, 2:3, :],
                                    sp_sb[0:64, 5:6], None, OP.mult)
            nc.vector.tensor_scalar(xcat[:, 65:66, 1:129],
                                    halo32[:, 1, 0:1, :],
                                    sp_sb[0:64, 4:5], None, OP.mult)

            om_T = fldA.tile([128, 64, 27], bf16, tag="omT")
            for c in range(16):  # chunks of 4 output rows
                ps = pom.tile([27, 512], f32)
                for k in range(9):
                    ky, kx = k // 3, k % 3
                    rhs = xcat[:, 4 * c + ky:4 * c + ky + 4, kx:kx + 128]
                    nc.tensor.matmul(ps, ow_sb[:, k, :], rhs,
                                     start=(k == 0), stop=(k == 8))
                omf_ch = omp.tile([27, 4, 128], bf16, tag="omfch")
                # int8 -> bf16 integers (proven DVE copy), then scale by
                # the per-channel quant step from spack col 2
                nc.vector.tensor_copy(omf_ch, omf_sb[:, 4 * c:4 * c + 4, :])
                nc.vector.tensor_scalar(omf_ch, omf_ch, sp_sb[0:27, 2:3],
                                        None, OP.mult)
                om_ch = omp.tile([27, 4, 128], bf16)
                nc.vector.tensor_tensor(
                    out=om_ch, in0=ps.rearrange("p (y x) -> p y x", y=4),
                    in1=omf_ch, op=OP.add)
                pt = ptr.tile([128, 4, 28], bf16)
                for j in range(4):
                    nc.tensor.transpose(pt[:, j, 0:27], om_ch[:, j, :],
                                        ident[0:27, 0:27])
                cp(om_T[:, 4 * c:4 * c + 4, :], pt[:, :, 0:27])

            # fields: cym [x, 9, 5, 64] (mask folded), cx [x, 9, 5, 64]
            sg = fldA.tile([128, 9, 64], bf16, tag="sg")
            nc.scalar.activation(
                sg, om_T[:, :, 18:27].rearrange("x y k -> x k y"), AF.Sigmoid)
            cym = fldA.tile([128, 9, NS, 64], bf16, tag="cym")
            dy_ap = om_T[:, :, 0:18:2].rearrange("x y k -> x k y")
            nc.vector.tensor_tensor(out=cym, in0=bcast(dy_ap, NS, 2),
                                    in1=bcast(syc, 9, 1), op=OP.subtract)
            nc.scalar.activation(cym, cym, AF.Abs)
            nc.vector.tensor_scalar(cym, cym, -1.0, 1.0, OP.mult, OP.add)
            nc.vector.tensor_scalar(cym, cym, 0.0, None, OP.max)
            nc.vector.tensor_tensor(out=cym, in0=cym, in1=bcast(sg, NS, 2),
                                    op=OP.mult)
            cx = fldA.tile([128, 9, NS, 64], bf16, tag="cx")
            dx_ap = om_T[:, :, 1:18:2].rearrange("x y k -> x k y")
            nc.vector.tensor_tensor(out=cx, in0=bcast(dx_ap, NS, 2),
                                    in1=bcast(sxc, 9, 1), op=OP.subtract)
            nc.scalar.activation(cx, cx, AF.Abs)
            nc.vector.tensor_scalar(cx, cx, -1.0, 1.0, OP.mult, OP.add)
            nc.vector.tensor_scalar(cx, cx, 0.0, None, OP.max)
            # Bf[x, k, sx, sy, y] = cx * cym (coefficients at the OUTPUT pixel)
            Bf = bfp.tile([128, 9, NS, NS, 64], bf16)
            nc.vector.tensor_tensor(out=Bf, in0=bcast(cx, NS, 3),
                                    in1=bcast(cym, NS, 2), op=OP.mult)
            phase1.close()

            # ---- phase 2: g = per-tap 1x1 conv on padded rows ----
            stackA = contextlib.ExitStack()
            gp = stackA.enter_context(tc.tile_pool(name="gp", bufs=1))
            featp = contextlib.ExitStack()
            fpool = featp.enter_context(tc.tile_pool(name="fpool", bufs=1))
            pg = featp.enter_context(tc.tile_pool(name="pg", bufs=2,
                                                  space="PSUM"))
            feat_i8 = fpool.tile([64, 64, 128], mybir.dt.int8)
            nc.sync.dma_start(
                out=feat_i8,
                in_=dat_d[:, 0:8192].rearrange("p (y x) -> p y x", y=64))
            feat_sb = fpool.tile([64, 70, 128], bf16)
            # int8 -> bf16 integers; f1 quant step folded into wk;
            # rows 0:3 / 67:70 come from the exchanged halos (masked to
            # exact zeros at the image boundary)
            nc.vector.tensor_copy(feat_sb[:, 3:67, :], feat_i8)
            nc.vector.tensor_scalar(feat_sb[:, 0:3, :], halo32[:, 0, :, :],
                                    sp_sb[0:64, 5:6], None, OP.mult)
            nc.vector.tensor_scalar(feat_sb[:, 67:70, :], halo32[:, 1, :, :],
                                    sp_sb[0:64, 4:5], None, OP.mult)

            g = gp.tile([128, 9, 64, 70], bf16)
            ga = g[:]

            def gdst(r):
                # [(2 chunks), 288 (k,o)-cols] view of g[:, :, :, r]
                return bass.AP(tensor=ga.tensor, offset=ga.offset + r,
                               ap=[list(ga.ap[0]), [20160, 2], [70, 288]])

            for r in range(70):
                psg = pg.tile([128, 2, 512], f32)
                nc.tensor.matmul(psg[:, 0, 0:288], feat_sb[:, r, :],
                                 wk_sb[:, 0:288], start=True, stop=True)
                nc.tensor.matmul(psg[:, 1, 0:288], feat_sb[:, r, :],
                                 wk_sb[:, 288:576], start=True, stop=True)
                cp(gdst(r), psg[:, 0:2, 0:288])
            featp.close()

            # ---- phase 3: flat warp sum over shifted-g planes ----
            # acc[x, o, y] = sum_{k,sx,sy} Bf[x,k,sx,sy,y]
            #                  * g[x+dlt, k, o, y+ky+sy],  dlt = kx-1+sx.
            # x-shifts of g via partition-shifted SBUF DMAs (per dlt, kx
            # plane group); 5 sy taps fused per op via a sliding-window AP
            # then reduced; o processed in halves to bound tmp size.
            gsp = stackA.enter_context(tc.tile_pool(name="gsp", bufs=1))
            acc = outp.tile([128, 64, 64], bf16)
            Gs = gsp.tile([128, 3, 64, 70], bf16)

            def ywin(ap, off):
                # [.., n(stride 1)] -> [.., 64, 5] sliding window at +off
                new = [list(p) for p in ap.ap[:-1]] + [[1, 64], [1, 5]]
                return bass.AP(tensor=ap.tensor, offset=ap.offset + off,
                               ap=new)

            first = [True]

            def warp_terms(slc, delta, kx):
                # slc(ky): [128, 64(o), 70(y')] plane for this kx
                sxi = delta - kx + 3
                for ky in range(3):
                    k = 3 * ky + kx
                    gw = ywin(slc(ky), ky)
                    bf_ap = bcast(Bf[:, k, sxi, :, :], 64, 1).rearrange(
                        "x o s y -> x o y s")
                    tmp = tmpp.tile([128, 64, 64, NS], bf16, tag="t")
                    nc.vector.tensor_tensor(out=tmp, in0=gw, in1=bf_ap,
                                            op=OP.mult)
                    tmp2 = tmpp.tile([128, 64, 64], f32, tag="t2")
                    nc.vector.tensor_reduce(tmp2, tmp,
                                            axis=mybir.AxisListType.X,
                                            op=OP.add)
                    if first[0]:
                        nc.vector.tensor_copy(acc, tmp2)
                        first[0] = False
                    else:
                        nc.vector.tensor_tensor(out=acc, in0=acc, in1=tmp2,
                                                op=OP.add)

            for kx in range(3):
                warp_terms(
                    lambda ky, kx=kx: g[:, 3 * ky + kx, :, :], 0, kx)
            for delta in (-3, -2, -1, 1, 2, 3):
                # quadrant-aligned memset band once per delta; the shift
                # DMAs only ever write the interior, so the edge stays zero
                # across the kx iterations.
                if delta > 0:
                    nc.vector.memset(Gs[96:128, :, :, :], 0.0)
                else:
                    nc.vector.memset(Gs[0:32, :, :, :], 0.0)
                for kx in range(max(0, delta - 1), min(2, delta + 3) + 1):
                    if delta > 0:
                        nc.sync.dma_start(
                            out=Gs[0:128 - delta, :, :, :],
                            in_=g[delta:128, kx:9:3, :, :])
                    else:
                        d = -delta
                        nc.sync.dma_start(
                            out=Gs[d:128, :, :, :],
                            in_=g[0:128 - d, kx:9:3, :, :])
                    warp_terms(lambda ky: Gs[:, ky, :, :], delta, kx)

            stackA.close()  # free g + Gs

            # ---- transpose acc -> hacc [(par,y), j, x] ----
            hp = ctx.enter_context(tc.tile_pool(name="hp", bufs=1))
            pv = ctx.enter_context(tc.tile_pool(name="pv", bufs=2,
                                                space="PSUM"))
            pst = ctx.enter_context(tc.tile_pool(name="pst", bufs=1,
                                                 space="PSUM"))
            hacc = hp.tile([128, 32, 128], bf16)
            for j2 in range(4):
                pvt = pv.tile([128, 8, 128], bf16)
                for jj in range(8):
                    j = 8 * j2 + jj
                    nc.tensor.transpose(
                        pvt[:, jj, :],
                        acc[:, 2 * j:2 * j + 2, :].rearrange(
                            "x o y -> x (o y)"),
                        ident)
                cp(hacc[:, 8 * j2:8 * j2 + 8, :], pvt)

            # ---- BN stats ----
            sq = hp.tile([128, 32, 128], bf16, tag="sq")
            nc.vector.tensor_tensor(out=sq, in0=hacc, in1=hacc, op=OP.mult)
            stat2 = fld.tile([128, 2, 32], f32, tag="st2")
            nc.vector.tensor_reduce(stat2[:, 0, :], hacc,
                                    axis=mybir.AxisListType.X, op=OP.add)
            nc.vector.tensor_reduce(stat2[:, 1, :], sq,
                                    axis=mybir.AxisListType.X, op=OP.add)
            ps1 = pst.tile([2, 2, 32], f32)
            nc.tensor.matmul(ps1.rearrange("p a b -> p (a b)"), sp_sb[:, 0:2],
                             stat2.rearrange("p a b -> p (a b)"),
                             start=True, stop=True)
            st_sb = fld.tile([2, 2, 32], f32, tag="stsb")
            nc.vector.tensor_copy(st_sb, ps1)
            cc_in = dram.tile([2, 2, 32], f32)
            cc_out = dram.tile([2, 2, 32], f32)
            nc.sync.dma_start(out=cc_in[:], in_=st_sb)
            nc.gpsimd.collective_compute(
                "AllReduce", OP.add,
                replica_groups=[list(range(N_CORES))],
                ins=[cc_in[:]], outs=[cc_out[:]])
            red = fld.tile([2, 2, 32], f32, tag="red")
            nc.sync.dma_start(out=red, in_=cc_out[:])

            mt = fld.tile([2, 32], f32, tag="mt")
            nc.vector.tensor_scalar(mt, red[:, 0, :], 1.0 / BN_N, None,
                                    OP.mult)
            ex2 = fld.tile([2, 32], f32, tag="ex2")
            nc.vector.tensor_scalar(ex2, red[:, 1, :], 1.0 / BN_N, None,
                                    OP.mult)
            var = fld.tile([2, 32], f32, tag="var")
            nc.vector.tensor_tensor(out=var, in0=mt, in1=mt, op=OP.mult)
            nc.vector.tensor_tensor(out=var, in0=ex2, in1=var, op=OP.subtract)
            nc.vector.tensor_scalar(var, var, EPS, None, OP.add)
            sqv = fld.tile([2, 32], f32, tag="sqv")
            nc.scalar.activation(sqv, var, AF.Sqrt)
            rstd = fld.tile([2, 32], f32, tag="rstd")
            nc.vector.reciprocal(rstd, sqv)
            AB = fld.tile([2, 2, 32], f32, tag="AB")
            nc.vector.tensor_tensor(out=AB[:, 0, :], in0=gb_sb[:, 0, :],
                                    in1=rstd, op=OP.mult)
            nc.vector.tensor_tensor(out=AB[:, 1, :], in0=mt, in1=AB[:, 0, :],
                                    op=OP.mult)
            nc.vector.tensor_tensor(out=AB[:, 1, :], in0=gb_sb[:, 1, :],
                                    in1=AB[:, 1, :], op=OP.subtract)
            ab_d = dram.tile([2, 2, 32], f32)
            nc.sync.dma_start(out=ab_d[:], in_=AB)
            ABc = fld.tile([128, 2, 32], f32, tag="ABc")
            nc.sync.dma_start(
                out=ABc,
                in_=bass.AP(tensor=ab_d.tensor, offset=ab_d.offset,
                            ap=[[64, 2], [0, 64], [32, 2], [1, 32]]))

            # ---- BN apply + int8 quantize + store (one DMA out) ----
            # gamma/beta are pre-divided by OUT_SCALE on the host, so
            # fin = hacc*A + B is already in quant units; clamp to the
            # int8 range (avoids wraparound on the ~1e-5 tail), convert.
            fin = hp.tile([128, 32, 128], f32)
            nc.vector.tensor_tensor(out=fin, in0=hacc,
                                    in1=bcast(ABc[:, 0, :], 128, 2),
                                    op=OP.mult)
            nc.vector.tensor_tensor(out=fin, in0=fin,
                                    in1=bcast(ABc[:, 1, :], 128, 2),
                                    op=OP.add)
            nc.vector.tensor_scalar(fin, fin, 127.0, None, OP.min)
            nc.vector.tensor_scalar(fin, fin, -127.0, None, OP.max)
            finq = hp.tile([128, 32, 128], mybir.dt.int8, tag="finq")
            nc.vector.tensor_copy(finq, fin)
            od = out_d[:]
            out_ap = bass.AP(tensor=od.tensor, offset=od.offset,
                             ap=[[8192, 2], [128, 64], [16384, 32], [1, 128]])
            nc.sync.dma_start(out=out_ap, in_=finq)

    nc.finalize()

    # The BIR debug_table embeds absolute source paths, which would make
    # the NEFF compile cache path-dependent (a fresh checkout would pay a
    # ~76s recompile). Normalize them in the serialized module.
    import re
    orig_to_json_bytes = nc.to_json_bytes

    def to_json_bytes_scrubbed():
        return re.sub(rb'"filename":"[^"]*"', b'"filename":"kernel.py"',
                      orig_to_json_bytes())

    nc.to_json_bytes = to_json_bytes_scrubbed
    return nc


_module_cache = {}


def get_module():
    if "m" not in _module_cache:
        _module_cache["m"] = build_module()
    return _module_cache["m"]


def _quant_i8(x, scale):
    # round(x/scale) clipped to [-127,127] in 4 passes:
    # t = clip(x/s, ...) + 127.5; floor via uint8 cast; -127 with wraparound
    # (uint8 mod-256 arithmetic == int8 two's complement).
    t = np.multiply(x, 1.0 / scale, dtype=np.float32)
    np.clip(t, -127.0, 127.0, out=t)
    t += 127.5
    q = t.astype(np.uint8)
    q -= 127
    return q.view(np.int8)


def _om_f3_host(f3, ow_f3, ob):
    """conv3x3(f3, ow[:, 64:]) + ob -> [4, 27, 128, 128] f32, via 36
    row-blocked skinny GEMMs on the zero-padded image."""
    B = f3.shape[0]
    Xp = np.zeros((B, 64, 130, 130), np.float32)
    Xp[:, :, 1:129, 1:129] = f3
    om = np.broadcast_to(ob[None, :, None, None], (B, 27, 128, 128)).copy()
    for ky in range(3):
        for kx in range(3):
            Wk = np.ascontiguousarray(ow_f3[:, :, ky, kx])  # [27, 64]
            for b in range(B):
                blk = Xp[b, :, ky:ky + 128, :].reshape(64, 128 * 130)
                t = (Wk @ blk).reshape(27, 128, 130)
                om[b] += t[:, :, kx:kx + 128]
    return om


def prep_global(f1_feat, f3_feat, offset_w, offset_b, main_w, gamma, beta):
    """Host-side packing into the CONCATENATED (8*shape0) global arrays
    that the sharded runner consumes directly."""
    bf = ml_dtypes.bfloat16
    f1q = _quant_i8(np.asarray(f1_feat, np.float32), F1_SCALE)  # [4,64,128,128]
    ow = np.asarray(offset_w, np.float32)   # [27,128,3,3]
    ob = np.asarray(offset_b, np.float32)
    wk = np.asarray(main_w, np.float32)     # [64,64,3,3]

    # merged per-core upload [64, 12288] int8: f1 INTERIOR rows
    # y0..y0+63 in cols 0:8192 (halos exchanged on device), the packed
    # omf in cols 8192:12288 (see build_module).
    dat = np.empty((N_CORES, 64, 12288), np.int8)
    dv = dat[:, :, 0:8192].reshape(N_CORES, 64, 64, 128)
    for i in range(N_CORES):
        b, half = i // 2, i % 2
        dv[i] = f1q[b][:, 64 * half:64 * half + 64, :]

    # f3 enters only through the 27-channel offset conv; computing that
    # contribution host-side shrinks its upload 64ch -> 27ch. Quantize
    # per-channel (scales land in spack col 2).
    omf = _om_f3_host(np.asarray(f3_feat, np.float32), ow[:, 64:128], ob)
    sc = np.maximum(np.abs(omf).max(axis=(0, 2, 3)) / 127.0, 1e-30)  # [27]
    omq = np.clip(np.rint(omf / sc[None, :, None, None]), -127, 127) \
        .astype(np.int8)
    for i in range(N_CORES):
        b, half = i // 2, i % 2
        # [27,64,128] -> [27,2,4096] -> partitions 2q+h
        dat[i, 0:54, 8192:12288] = \
            omq[b][:, 64 * half:64 * half + 64, :].reshape(54, 4096)
        dat[i, 54:64, 8192:12288] = 0

    # wpack: ow_t f1-half [64,243] (rows 64:128 unused) | wk packed
    # [128,288] | ident [128,128]; int8 quant steps folded in
    ow_t = np.zeros((128, 243), np.float32)
    ow_t[0:64] = ow[:, 0:64].reshape(27, 64, 9).transpose(1, 2, 0) \
        .reshape(64, 243) * F1_SCALE
    wk_t = wk.reshape(64, 64, 9).transpose(1, 2, 0).reshape(64, 576) * F1_SCALE
    wk_r = np.concatenate([wk_t[:, 0:288], wk_t[:, 288:576]], axis=0)
    wpack = np.concatenate(
        [ow_t, wk_r, np.eye(128, dtype=np.float32)], axis=1).astype(bf)

    # spack: sel cols 0-1 | omf per-channel scales col 2 | gb flat col 3
    # | halo role masks cols 4-5 (per-core: even, odd)
    spack = np.zeros((128, 6), np.float32)
    spack[0:64, 0] = 1.0
    spack[64:128, 1] = 1.0
    spack[0:27, 2] = sc
    # pre-divide gamma/beta by OUT_SCALE so the on-device BN affine lands
    # directly in int8 quant units
    gam = np.asarray(gamma, np.float32) / OUT_SCALE
    bet = np.asarray(beta, np.float32) / OUT_SCALE
    gb = np.zeros((2, 2, 32), np.float32)
    for par in range(2):
        gb[par, 0, :] = gam[par::2]
        gb[par, 1, :] = bet[par::2]
    spack[:, 3] = gb.reshape(-1)
    # per-core role masks for the halo exchange
    spack_all = np.tile(spack, (N_CORES, 1)).reshape(N_CORES, 128, 6)
    for i in range(N_CORES):
        spack_all[i, :, 4] = 1.0 if i % 2 == 0 else 0.0
        spack_all[i, :, 5] = 0.0 if i % 2 == 0 else 1.0

    # wpack is per-core [128, ...] (tiled to the global layout only on
    # rare weight re-upload); spack carries per-core masks so it ships
    # as the full (8*128, 6) array.
    return (dat.reshape(N_CORES * 64, 12288), wpack,
            spack_all.reshape(N_CORES * 128, 6))


class _AxonRunner:
    """Persistent PJRT runner: one shard_map jit, resident weights,
    donated output scratch chained from the previous call."""

    def __init__(self, nc):
        import jax
        import warnings
        from jax.sharding import Mesh, PartitionSpec, NamedSharding
        with warnings.catch_warnings():
            warnings.simplefilter("ignore")
            from jax.experimental.shard_map import shard_map
        from concourse.bass2jax import (
            _bass_exec_p, install_neuronx_cc_hook, partition_id_tensor)

        install_neuronx_cc_hook()
        self.jax = jax
        self.nc = nc

        partition_name = (nc.partition_id_tensor.name
                          if nc.partition_id_tensor else None)
        in_names, out_names, out_avals = [], [], []
        for alloc in nc.m.functions[0].allocations:
            if not isinstance(alloc, mybir.MemoryLocationSet):
                continue
            name = alloc.memorylocations[0].name
            if alloc.kind == "ExternalInput":
                if name != partition_name:
                    in_names.append(name)
            elif alloc.kind == "ExternalOutput":
                out_names.append(name)
                out_avals.append(jax.core.ShapedArray(
                    tuple(alloc.tensor_shape), mybir.dt.np(alloc.dtype)))
        assert in_names == ["dat", "wpack", "spack"], in_names
        assert out_names == ["out"], out_names
        n_params = len(in_names)
        n_outs = len(out_avals)
        all_names = list(in_names) + list(out_names)
        if partition_name is not None:
            all_names.append(partition_name)

        def _body(*args):
            operands = list(args)
            if partition_name is not None:
                operands.append(partition_id_tensor())
            outs = _bass_exec_p.bind(
                *operands, out_avals=tuple(out_avals),
                in_names=tuple(all_names), out_names=tuple(out_names),
                lowering_input_output_aliases=(),
                sim_require_finite=True, sim_require_nnan=True, nc=nc)
            return tuple(outs)

        devices = jax.devices()[:N_CORES]
        mesh = Mesh(np.asarray(devices), ("core",))
        self.sh = NamedSharding(mesh, PartitionSpec("core"))
        self.jfn = jax.jit(
            shard_map(_body, mesh=mesh,
                      in_specs=(PartitionSpec("core"),) * (n_params + n_outs),
                      out_specs=(PartitionSpec("core"),) * n_outs,
                      check_rep=False),
            donate_argnums=tuple(range(n_params, n_params + n_outs)),
            keep_unused=True)

        self.w_key = None
        self.w_dev = None   # (wpack_dev, spack_dev)
        self.scratch = None

    def __call__(self, dat_all, wpack, spack_all):
        jax = self.jax
        # weights: resident unless their bytes change (1.4MB -> the
        # hash check is ~0.5ms); spack_all is already the full global
        key = (wpack.tobytes(), spack_all.tobytes())
        if self.w_key != key:
            self.w_dev = (jax.device_put(np.tile(wpack, (N_CORES, 1)),
                                         self.sh),
                          jax.device_put(spack_all, self.sh))
            self.w_key = key
        if self.scratch is None:
            self.scratch = jax.device_put(
                np.zeros((N_CORES * 64, 64, 128), np.int8), self.sh)
        dat_dev = jax.device_put(dat_all, self.sh)
        outs = self.jfn(dat_dev, self.w_dev[0], self.w_dev[1], self.scratch)
        out = outs[0]
        # the kernel writes every element of out, so the previous output
        # buffer is a valid scratch donation for the next call (the caller
        # materializes host copies before the next call happens)
        self.scratch = out
        return out  # global jax array [8*64, 64, 128] int8


def _get_runner():
    if "r" not in _module_cache:
        _module_cache["r"] = _AxonRunner(get_module())
    return _module_cache["r"]


def run_device(dat_all, wpack, spack):
    """One full device round: upload dat, run 8 cores, download +
    unpack to the final [4, 64, 128, 128] f32 output."""
    from concourse._compat import axon_active
    out = np.empty((4, 64, 128, 128), np.float32)
    s = np.float32(OUT_SCALE)
    if axon_active():
        res = _get_runner()(dat_all, wpack, spack)  # [512,64,128] i8
        # fetch per-shard and dequantize each as it lands, so the host
        # unpack overlaps the tail of the tunnel download
        by_row = {}
        for sd in res.addressable_shards:
            sd.data.copy_to_host_async()
            by_row[sd.index[0].start or 0] = sd.data
        for i in range(N_CORES):
            b, h = i // 2, i % 2
            np.multiply(np.asarray(by_row[64 * i]), s,
                        out=out[b, :, 64 * h:64 * h + 64, :],
                        casting="unsafe")
        return out
    nc = get_module()
    maps = [{"dat": dat_all[64 * i:64 * i + 64], "wpack": wpack,
             "spack": spack[128 * i:128 * i + 128]}
            for i in range(N_CORES)]
    rr = run_bass_kernel_spmd(nc, maps, core_ids=list(range(N_CORES)))
    for i in range(N_CORES):
        b, h = i // 2, i % 2
        np.multiply(rr.results[i]["out"], s,
                    out=out[b, :, 64 * h:64 * h + 64, :], casting="unsafe")
    return out


def _bn_ok(out, gamma, beta):
    """BN output invariant: per-channel mean==beta, std≈|gamma| (batch
    statistics are computed from this very tensor). Good runs deviate
    <5e-4; a corrupted round (rare tunnel/device glitch) trips this."""
    g = np.abs(np.asarray(gamma, np.float32))
    b = np.asarray(beta, np.float32)
    ref = np.maximum(g, 1e-3)
    m = out.mean(axis=(0, 2, 3))
    s = out.std(axis=(0, 2, 3))
    return bool((np.abs(m - b) <= 0.02 * ref).all()
                and (np.abs(s - g) <= 0.03 * ref).all())


def _kernel_subprocess(**inputs):
    """Rerun in a fresh process. The device sporadically hard-crashes
    (NRT_EXEC_UNIT_UNRECOVERABLE, ~1/300 rounds) which poisons the PJRT
    client for the whole process; a fresh process recovers (NEFF compile
    is disk-cached, so this costs ~3s)."""
    import subprocess
    import sys
    import tempfile
    me = os.path.abspath(__file__)
    with tempfile.TemporaryDirectory() as td:
        np.savez(os.path.join(td, "in.npz"),
                 **{k: np.asarray(v) for k, v in inputs.items()})
        code = (
            "import os, numpy as np, importlib.util\n"
            "os.environ['KERNEL_NO_SUBPROC'] = '1'\n"
            f"spec = importlib.util.spec_from_file_location('kmod', {me!r})\n"
            "m = importlib.util.module_from_spec(spec)\n"
            "spec.loader.exec_module(m)\n"
            f"d = np.load(os.path.join({td!r}, 'in.npz'))\n"
            "out = m.kernel(**{k: d[k] for k in d.files})\n"
            f"np.save(os.path.join({td!r}, 'out.npy'), out)\n")
        subprocess.run([sys.executable, "-c", code], check=True)
        return np.load(os.path.join(td, "out.npy"))


def kernel(**inputs):
    packed = prep_global(**inputs)
    try:
        out = run_device(*packed)
        for _ in range(2):
            if _bn_ok(out, inputs["gamma"], inputs["beta"]):
                break
            r = _module_cache.get("r")
            if r is not None:
                r.w_key = None  # force weight re-upload on the retry
            out = run_device(*packed)
        return out
    except Exception:
        if os.environ.get("KERNEL_NO_SUBPROC"):
            raise
        return _kernel_subprocess(**inputs)


if __name__ == "__main__":
    d = np.load("/root/problem/ref_cache.npz")
    inp = {k: d[k] for k in d.files if k != "expected"}
    got = kernel(**inp)
    exp = d["expected"]
    err = np.linalg.norm(got - exp) / np.linalg.norm(exp)
    print("rel l2 err:", err, "maxabs:", np.abs(got - exp).max())


# revision 59
# speedup vs baseline: 1.0939x; 1.0634x over previous
"""Trainium2 Bass kernel for nn_DeformableAlignment.

Sharding: 8 cores = (batch b in 0..4) x (image row-half in {0,1}).
Each core computes out[b, :, y0:y0+64, :] for y0 = 64*(i%2).

Math (per core, matches reference exactly):
  om  = conv3x3(f1-half on device) + omf (f3-half, host) [27, 64, 128]
  dy/dx per tap k; sg = sigmoid(mask-channels)
  bilinear warp written floor-free via hat fields at the OUTPUT pixel:
    Bf[k,sx,sy] = relu(1-|dx-sx|) * relu(1-|dy-sy|)*sg  (sx,sy in -2..2)
  g[k] = 1x1-conv of f1 with main_w tap k, computed ONCE on the
         y-padded grid: g[x=128 part, 9k, 64o, 70y]     (140 matmuls)
  acc[x,o,y] = sum_{k,sx,sy} Bf[x,k,sx,sy,y] * g[x+dlt, k, o, y+ky+sy],
         dlt = kx-1+sx: x-shifts of g are 12 partition-shifted
         SBUF->SBUF DMAs (per dlt & kx plane); the 5 sy taps are fused
         per op via a sliding-window AP + reduce.
  BN stats via on-device partial sums + AllReduce across 8 cores; the
  BN affine is pre-divided by OUT_SCALE so the output quantizes to int8.

IO is minimized for the axon tunnel (aggregate ~45-70MB/s shared by
both directions, ~47ms dispatch RTT that pipelines away, and ~50ms of
NON-pipelined fixed cost per device_put -> everything rides ONE tensor):
  dat   [64, 12288] int8, merged per-call upload:
        cols 0:8192     f1 INTERIOR rows y0..y0+63 as [64,128],
                        quantized at 4/127 (clip 4sig; scale folded
                        into ow/wk host-side, so ints convert straight
                        to bf16). The 3-row warp halos are NOT
                        uploaded: the row-half pair cores (2b, 2b+1)
                        exchange them via a masked pair AllReduce
                        (role masks in spack cols 4-5 keep the SPMD
                        program branchless; re-masking at read time
                        yields exact zeros at the image boundary).
        cols 8192:12288 omf = the f3 half of the offset conv (+bias),
                        computed host-side (f3 only feeds this
                        27-channel conv, so shipping the contracted
                        result cuts that upload 64ch -> 27ch); channel
                        q's 8192B split across partitions 2q/2q+1,
                        un-permuted on device by one affine DMA;
                        per-channel scales ride in spack col 2.
  wpack [128, 659] bf16: ow_t f1-half [64,243] | wk packed [128,288]
        | ident (ow/wk scaled by the f1 quant step)
  spack [128, 6] f32 per core: sel | sel | omf scales |
        (gamma,beta)/OUT_SCALE | mask_even | mask_odd
  out   [64, 64, 128] int8 (dequantized to f32 on host; BN output is
        ~N(0,1) per channel so a 4/127 step keeps rel err ~1.75%)

Runner: under axon, run_bass_kernel_spmd would rebuild a jax.jit
(re-trace + re-lower, embedding the multi-MB BIR) and re-upload
weights + donated zero output buffers on EVERY call.  _AxonRunner
instead keeps one persistent shard_map jit, keeps wpack/spack resident
on device (re-uploaded only if the weight bytes change), and donates
the PREVIOUS call's output buffer as the next call's scratch (the
kernel writes every output element, so the scratch contents are
irrelevant after the first call's zeros).  Per call the tunnel moves
one 6.3MB upload and one 4.2MB download.  kernel() self-checks the BN
output invariant (per-channel mean==beta, std==|gamma|) and retries
the rare corrupted round; on the sporadic hard device crash
(NRT_EXEC_UNIT_UNRECOVERABLE poisons the process's PJRT client) it
recomputes in a fresh subprocess.
"""

import os

# Source tracebacks embed absolute file paths in the BIR, which makes the
# NEFF compile cache path-dependent (a fresh checkout would recompile for
# ~76s) and slows compilation. Disable before the module is built.
os.environ.setdefault("BASS_DISABLE_FRAME_TO_TRACEBACK", "1")

import numpy as np
import ml_dtypes

import concourse.bass as bass
import concourse.bacc as bacc
import concourse.tile as tile
from concourse import mybir
from concourse.bass_utils import run_bass_kernel_spmd

f32 = mybir.dt.float32
bf16 = mybir.dt.bfloat16
AF = mybir.ActivationFunctionType
OP = mybir.AluOpType

N_CORES = 8
NS = 5  # shifts -2..2
OUT_SCALE = 4.0 / 127.0  # int8 output quant step (BN output is ~N(0,1))
F1_SCALE = 4.0 / 127.0   # int8 quant step for the f1 upload
F3_SCALE = 4.0 / 127.0   # int8 quant step for the f3 upload
EPS = 1e-5
BN_N = 4 * 128 * 128  # elements per channel for batch stats


def bcast(ap, n, dim):
    """Insert a broadcast (step-0) dim of size n at position dim."""
    new = [list(p) for p in ap.ap]
    new.insert(dim, [0, n])
    return bass.AP(tensor=ap.tensor, offset=ap.offset, ap=new)


def build_module():
    nc = bacc.Bacc("TRN2", target_bir_lowering=False, debug=False,
                   num_devices=N_CORES)
    # ONE merged per-call upload (a second device_put costs ~50ms of
    # non-pipelined fixed overhead on the tunnel):
    #   cols 0:8192      = f1 INTERIOR rows y0..y0+63 as [64,128] int8
    #                      (the 3-row halos are exchanged between the
    #                      row-half pair cores via a masked AllReduce)
    #   cols 8192:12288  = omf (f3 offset-conv half): channel q's 8192
    #                      bytes as rows 0:32 on partition 2q and rows
    #                      32:64 on partition 2q+1; partitions 54:64 pad
    dat_d = nc.dram_tensor("dat", [64, 12288], mybir.dt.int8,
                           kind="ExternalInput")
    wp_d = nc.dram_tensor("wpack", [128, 659], bf16, kind="ExternalInput")
    # spack col 4 = mask_even (1 on even cores), col 5 = mask_odd
    sp_d = nc.dram_tensor("spack", [128, 6], f32, kind="ExternalInput")
    out_d = nc.dram_tensor("out", [64, 64, 128], mybir.dt.int8,
                           kind="ExternalOutput")

    import itertools
    cp_engines = itertools.cycle([0, 1])

    def cp(out, in_):
        if next(cp_engines) == 0:
            nc.vector.tensor_copy(out, in_)
        else:
            nc.scalar.copy(out, in_)

    with tile.TileContext(nc) as tc:
        import contextlib
        ctx = contextlib.ExitStack()
        with ctx:
            const = ctx.enter_context(tc.tile_pool(name="const", bufs=1))
            fld = ctx.enter_context(tc.tile_pool(name="fld", bufs=1))
            dram = ctx.enter_context(tc.tile_pool(name="dram", bufs=1,
                                                  space="DRAM"))
            tmpp = ctx.enter_context(tc.tile_pool(name="tmpp", bufs=1))
            outp = ctx.enter_context(tc.tile_pool(name="outp", bufs=1))
            bfp = ctx.enter_context(tc.tile_pool(name="bfp", bufs=1))

            # ---- constants in ----
            ow_sb = const.tile([64, 9, 27], bf16)
            nc.sync.dma_start(out=ow_sb, in_=wp_d[0:64, 0:243])
            wk_sb = const.tile([64, 576], bf16)
            nc.sync.dma_start(out=wk_sb[:, 0:288], in_=wp_d[0:64, 243:531])
            nc.sync.dma_start(out=wk_sb[:, 288:576], in_=wp_d[64:128, 243:531])
            ident = const.tile([128, 128], bf16)
            nc.sync.dma_start(out=ident, in_=wp_d[:, 531:659])
            sp_sb = const.tile([128, 6], f32)
            nc.sync.dma_start(out=sp_sb, in_=sp_d[:])
            gb_sb = const.tile([2, 2, 32], f32)
            nc.sync.dma_start(out=gb_sb, in_=sp_d[:, 3:4])
            syc = const.tile([128, NS, 64], bf16)
            sxc = const.tile([128, NS, 64], bf16)
            for i in range(NS):
                nc.vector.memset(syc[:, i, :], float(i - 2))
                nc.vector.memset(sxc[:, i, :], float(i - 2))

            # ---- phase 1: offset conv + fields ----
            phase1 = contextlib.ExitStack()
            xcp = phase1.enter_context(tc.tile_pool(name="xcp", bufs=1))
            fldA = phase1.enter_context(tc.tile_pool(name="fldA", bufs=1))
            omp = phase1.enter_context(tc.tile_pool(name="omp", bufs=2))
            pom = phase1.enter_context(tc.tile_pool(name="pom", bufs=2,
                                                    space="PSUM"))
            ptr = phase1.enter_context(tc.tile_pool(name="ptr", bufs=2,
                                                    space="PSUM"))

            # f1 interior int8; freed with this pool at phase1.close()
            dat_sb = xcp.tile([64, 64, 128], mybir.dt.int8)
            nc.sync.dma_start(
                out=dat_sb,
                in_=dat_d[:, 0:8192].rearrange("p (y x) -> p y x", y=64))
            # host-computed f3 half of the offset conv (+bias), int8 with
            # per-channel scales in spack col 2; un-permute the packed
            # [54 part, 4096B] DRAM layout to [27 part, 8192B] in one
            # affine DMA (src partition = 2q+h)
            omf_sb = xcp.tile([27, 64, 128], mybir.dt.int8)
            dst = omf_sb[:]
            dd = dat_d[:]
            nc.sync.dma_start(
                out=bass.AP(tensor=dst.tensor, offset=dst.offset,
                            ap=[list(dst.ap[0]), [4096, 2], [1, 4096]]),
                in_=bass.AP(tensor=dd.tensor, offset=dd.offset + 8192,
                            ap=[[2 * 12288, 27], [12288, 2], [1, 4096]]))

            # ---- halo exchange between pair cores (2b, 2b+1) ----
            # slot0 = even core's interior rows 61..63 (odd's above halo),
            # slot1 = odd core's interior rows 0..2 (even's below halo);
            # role masks make the SPMD program branchless, and re-masking
            # at read time yields exact zeros at the image boundary.
            hx1 = omp.tile([64, 2, 3, 128], bf16, tag="hx1")
            nc.vector.tensor_copy(hx1[:, 0], dat_sb[:, 61:64, :])
            nc.vector.tensor_copy(hx1[:, 1], dat_sb[:, 0:3, :])
            hxm = omp.tile([64, 2, 3, 128], f32, tag="hxm")
            nc.vector.tensor_scalar(hxm[:, 0], hx1[:, 0],
                                    sp_sb[0:64, 4:5], None, OP.mult)
            nc.vector.tensor_scalar(hxm[:, 1], hx1[:, 1],
                                    sp_sb[0:64, 5:6], None, OP.mult)
            cch_in = dram.tile([64, 2, 3, 128], f32)
            cch_out = dram.tile([64, 2, 3, 128], f32)
            nc.sync.dma_start(out=cch_in[:], in_=hxm)
            nc.gpsimd.collective_compute(
                "AllReduce", OP.add,
                replica_groups=[[2 * b, 2 * b + 1] for b in range(4)],
                ins=[cch_in[:]], outs=[cch_out[:]])
            halo32 = fld.tile([64, 2, 3, 128], f32, tag="halo")
            nc.sync.dma_start(out=halo32, in_=cch_out[:])

            xcat = xcp.tile([64, 66, 130], bf16)
            nc.vector.memset(xcat, 0.0)
            # int8 -> bf16 integer conversion (quant step folded into ow);
            # rows: 0 = above-halo row y0-1, 1:65 = interior, 65 = y0+64
            nc.vector.tensor_copy(xcat[:, 1:65, 1:129], dat_sb)
            nc.vector.tensor_scalar(xcat[:, 0:1, 1:129], halo32[:, 0, 2:3, :],
                                    sp_sb[0:64, 5:6], None, OP.mult)
            nc.vector.tensor_scalar(xcat[:, 65:66, 1:129],
                                    halo32[:, 1, 0:1, :],
                                    sp_sb[0:64, 4:5], None, OP.mult)

            om_T = fldA.tile([128, 64, 27], bf16, tag="omT")
            for c in range(16):  # chunks of 4 output rows
                ps = pom.tile([27, 512], f32)
                for k in range(9):
                    ky, kx = k // 3, k % 3
                    rhs = xcat[:, 4 * c + ky:4 * c + ky + 4, kx:kx + 128]
                    nc.tensor.matmul(ps, ow_sb[:, k, :], rhs,
                                     start=(k == 0), stop=(k == 8))
                omf_ch = omp.tile([27, 4, 128], bf16, tag="omfch")
                # int8 -> bf16 integers (proven DVE copy), then scale by
                # the per-channel quant step from spack col 2
                nc.vector.tensor_copy(omf_ch, omf_sb[:, 4 * c:4 * c + 4, :])
                nc.vector.tensor_scalar(omf_ch, omf_ch, sp_sb[0:27, 2:3],
                                        None, OP.mult)
                om_ch = omp.tile([27, 4, 128], bf16)
                nc.vector.tensor_tensor(
                    out=om_ch, in0=ps.rearrange("p (y x) -> p y x", y=4),
                    in1=omf_ch, op=OP.add)
                pt = ptr.tile([128, 4, 28], bf16)
                for j in range(4):
                    nc.tensor.transpose(pt[:, j, 0:27], om_ch[:, j, :],
                                        ident[0:27, 0:27])
                cp(om_T[:, 4 * c:4 * c + 4, :], pt[:, :, 0:27])

            # fields: cym [x, 9, 5, 64] (mask folded), cx [x, 9, 5, 64]
            sg = fldA.tile([128, 9, 64], bf16, tag="sg")
            nc.scalar.activation(
                sg, om_T[:, :, 18:27].rearrange("x y k -> x k y"), AF.Sigmoid)
            cym = fldA.tile([128, 9, NS, 64], bf16, tag="cym")
            dy_ap = om_T[:, :, 0:18:2].rearrange("x y k -> x k y")
            nc.vector.tensor_tensor(out=cym, in0=bcast(dy_ap, NS, 2),
                                    in1=bcast(syc, 9, 1), op=OP.subtract)
            nc.scalar.activation(cym, cym, AF.Abs)
            nc.vector.tensor_scalar(cym, cym, -1.0, 1.0, OP.mult, OP.add)
            nc.vector.tensor_scalar(cym, cym, 0.0, None, OP.max)
            nc.vector.tensor_tensor(out=cym, in0=cym, in1=bcast(sg, NS, 2),
                                    op=OP.mult)
            cx = fldA.tile([128, 9, NS, 64], bf16, tag="cx")
            dx_ap = om_T[:, :, 1:18:2].rearrange("x y k -> x k y")
            nc.vector.tensor_tensor(out=cx, in0=bcast(dx_ap, NS, 2),
                                    in1=bcast(sxc, 9, 1), op=OP.subtract)
            nc.scalar.activation(cx, cx, AF.Abs)
            nc.vector.tensor_scalar(cx, cx, -1.0, 1.0, OP.mult, OP.add)
            nc.vector.tensor_scalar(cx, cx, 0.0, None, OP.max)
            # Bf[x, k, sx, sy, y] = cx * cym (coefficients at the OUTPUT pixel)
            Bf = bfp.tile([128, 9, NS, NS, 64], bf16)
            nc.vector.tensor_tensor(out=Bf, in0=bcast(cx, NS, 3),
                                    in1=bcast(cym, NS, 2), op=OP.mult)
            phase1.close()

            # ---- phase 2: g = per-tap 1x1 conv on padded rows ----
            stackA = contextlib.ExitStack()
            gp = stackA.enter_context(tc.tile_pool(name="gp", bufs=1))
            featp = contextlib.ExitStack()
            fpool = featp.enter_context(tc.tile_pool(name="fpool", bufs=1))
            pg = featp.enter_context(tc.tile_pool(name="pg", bufs=2,
                                                  space="PSUM"))
            feat_i8 = fpool.tile([64, 64, 128], mybir.dt.int8)
            nc.sync.dma_start(
                out=feat_i8,
                in_=dat_d[:, 0:8192].rearrange("p (y x) -> p y x", y=64))
            feat_sb = fpool.tile([64, 70, 128], bf16)
            # int8 -> bf16 integers; f1 quant step folded into wk;
            # rows 0:3 / 67:70 come from the exchanged halos (masked to
            # exact zeros at the image boundary)
            nc.vector.tensor_copy(feat_sb[:, 3:67, :], feat_i8)
            nc.vector.tensor_scalar(feat_sb[:, 0:3, :], halo32[:, 0, :, :],
                                    sp_sb[0:64, 5:6], None, OP.mult)
            nc.vector.tensor_scalar(feat_sb[:, 67:70, :], halo32[:, 1, :, :],
                                    sp_sb[0:64, 4:5], None, OP.mult)

            g = gp.tile([128, 9, 64, 70], bf16)
            ga = g[:]

            def gdst(r):
                # [(2 chunks), 288 (k,o)-cols] view of g[:, :, :, r]
                return bass.AP(tensor=ga.tensor, offset=ga.offset + r,
                               ap=[list(ga.ap[0]), [20160, 2], [70, 288]])

            for r in range(70):
                psg = pg.tile([128, 2, 512], f32)
                nc.tensor.matmul(psg[:, 0, 0:288], feat_sb[:, r, :],
                                 wk_sb[:, 0:288], start=True, stop=True)
                nc.tensor.matmul(psg[:, 1, 0:288], feat_sb[:, r, :],
                                 wk_sb[:, 288:576], start=True, stop=True)
                cp(gdst(r), psg[:, 0:2, 0:288])
            featp.close()

            # ---- phase 3: flat warp sum over shifted-g planes ----
            # acc[x, o, y] = sum_{k,sx,sy} Bf[x,k,sx,sy,y]
            #                  * g[x+dlt, k, o, y+ky+sy],  dlt = kx-1+sx.
            # x-shifts of g via partition-shifted SBUF DMAs (per dlt, kx
            # plane group); 5 sy taps fused per op via a sliding-window AP
            # then reduced; o processed in halves to bound tmp size.
            gsp = stackA.enter_context(tc.tile_pool(name="gsp", bufs=1))
            acc = outp.tile([128, 64, 64], bf16)
            Gs = gsp.tile([128, 3, 64, 70], bf16)

            def ywin(ap, off):
                # [.., n(stride 1)] -> [.., 64, 5] sliding window at +off
                new = [list(p) for p in ap.ap[:-1]] + [[1, 64], [1, 5]]
                return bass.AP(tensor=ap.tensor, offset=ap.offset + off,
                               ap=new)

            first = [True]

            def warp_terms(slc, delta, kx):
                # slc(ky): [128, 64(o), 70(y')] plane for this kx
                sxi = delta - kx + 3
                for ky in range(3):
                    k = 3 * ky + kx
                    gw = ywin(slc(ky), ky)
                    bf_ap = bcast(Bf[:, k, sxi, :, :], 64, 1).rearrange(
                        "x o s y -> x o y s")
                    tmp = tmpp.tile([128, 64, 64, NS], bf16, tag="t")
                    nc.vector.tensor_tensor(out=tmp, in0=gw, in1=bf_ap,
                                            op=OP.mult)
                    tmp2 = tmpp.tile([128, 64, 64], f32, tag="t2")
                    nc.vector.tensor_reduce(tmp2, tmp,
                                            axis=mybir.AxisListType.X,
                                            op=OP.add)
                    if first[0]:
                        nc.vector.tensor_copy(acc, tmp2)
                        first[0] = False
                    else:
                        nc.vector.tensor_tensor(out=acc, in0=acc, in1=tmp2,
                                                op=OP.add)

            for kx in range(3):
                warp_terms(
                    lambda ky, kx=kx: g[:, 3 * ky + kx, :, :], 0, kx)
            for delta in (-3, -2, -1, 1, 2, 3):
                # quadrant-aligned memset band once per delta; the shift
                # DMAs only ever write the interior, so the edge stays zero
                # across the kx iterations.
                if delta > 0:
                    nc.vector.memset(Gs[96:128, :, :, :], 0.0)
                else:
                    nc.vector.memset(Gs[0:32, :, :, :], 0.0)
                for kx in range(max(0, delta - 1), min(2, delta + 3) + 1):
                    if delta > 0:
                        nc.sync.dma_start(
                            out=Gs[0:128 - delta, :, :, :],
                            in_=g[delta:128, kx:9:3, :, :])
                    else:
                        d = -delta
                        nc.sync.dma_start(
                            out=Gs[d:128, :, :, :],
                            in_=g[0:128 - d, kx:9:3, :, :])
                    warp_terms(lambda ky: Gs[:, ky, :, :], delta, kx)

            stackA.close()  # free g + Gs

            # ---- transpose acc -> hacc [(par,y), j, x] ----
            hp = ctx.enter_context(tc.tile_pool(name="hp", bufs=1))
            pv = ctx.enter_context(tc.tile_pool(name="pv", bufs=2,
                                                space="PSUM"))
            pst = ctx.enter_context(tc.tile_pool(name="pst", bufs=1,
                                                 space="PSUM"))
            hacc = hp.tile([128, 32, 128], bf16)
            for j2 in range(4):
                pvt = pv.tile([128, 8, 128], bf16)
                for jj in range(8):
                    j = 8 * j2 + jj
                    nc.tensor.transpose(
                        pvt[:, jj, :],
                        acc[:, 2 * j:2 * j + 2, :].rearrange(
                            "x o y -> x (o y)"),
                        ident)
                cp(hacc[:, 8 * j2:8 * j2 + 8, :], pvt)

            # ---- BN stats ----
            sq = hp.tile([128, 32, 128], bf16, tag="sq")
            nc.vector.tensor_tensor(out=sq, in0=hacc, in1=hacc, op=OP.mult)
            stat2 = fld.tile([128, 2, 32], f32, tag="st2")
            nc.vector.tensor_reduce(stat2[:, 0, :], hacc,
                                    axis=mybir.AxisListType.X, op=OP.add)
            nc.vector.tensor_reduce(stat2[:, 1, :], sq,
                                    axis=mybir.AxisListType.X, op=OP.add)
            ps1 = pst.tile([2, 2, 32], f32)
            nc.tensor.matmul(ps1.rearrange("p a b -> p (a b)"), sp_sb[:, 0:2],
                             stat2.rearrange("p a b -> p (a b)"),
                             start=True, stop=True)
            st_sb = fld.tile([2, 2, 32], f32, tag="stsb")
            nc.vector.tensor_copy(st_sb, ps1)
            cc_in = dram.tile([2, 2, 32], f32)
            cc_out = dram.tile([2, 2, 32], f32)
            nc.sync.dma_start(out=cc_in[:], in_=st_sb)
            nc.gpsimd.collective_compute(
                "AllReduce", OP.add,
                replica_groups=[list(range(N_CORES))],
                ins=[cc_in[:]], outs=[cc_out[:]])
            red = fld.tile([2, 2, 32], f32, tag="red")
            nc.sync.dma_start(out=red, in_=cc_out[:])

            mt = fld.tile([2, 32], f32, tag="mt")
            nc.vector.tensor_scalar(mt, red[:, 0, :], 1.0 / BN_N, None,
                                    OP.mult)
            ex2 = fld.tile([2, 32], f32, tag="ex2")
            nc.vector.tensor_scalar(ex2, red[:, 1, :], 1.0 / BN_N, None,
                                    OP.mult)
            var = fld.tile([2, 32], f32, tag="var")
            nc.vector.tensor_tensor(out=var, in0=mt, in1=mt, op=OP.mult)
            nc.vector.tensor_tensor(out=var, in0=ex2, in1=var, op=OP.subtract)
            nc.vector.tensor_scalar(var, var, EPS, None, OP.add)
            sqv = fld.tile([2, 32], f32, tag="sqv")
            nc.scalar.activation(sqv, var, AF.Sqrt)
            rstd = fld.tile([2, 32], f32, tag="rstd")
            nc.vector.reciprocal(rstd, sqv)
            AB = fld.tile([2, 2, 32], f32, tag="AB")
            nc.vector.tensor_tensor(out=AB[:, 0, :], in0=gb_sb[:, 0, :],
                                    in1=rstd, op=OP.mult)
            nc.vector.tensor_tensor(out=AB[:, 1, :], in0=mt, in1=AB[:, 0, :],
                                    op=OP.mult)
            nc.vector.tensor_tensor(out=AB[:, 1, :], in0=gb_sb[:, 1, :],
                                    in1=AB[:, 1, :], op=OP.subtract)
            ab_d = dram.tile([2, 2, 32], f32)
            nc.sync.dma_start(out=ab_d[:], in_=AB)
            ABc = fld.tile([128, 2, 32], f32, tag="ABc")
            nc.sync.dma_start(
                out=ABc,
                in_=bass.AP(tensor=ab_d.tensor, offset=ab_d.offset,
                            ap=[[64, 2], [0, 64], [32, 2], [1, 32]]))

            # ---- BN apply + int8 quantize + store (one DMA out) ----
            # gamma/beta are pre-divided by OUT_SCALE on the host, so
            # fin = hacc*A + B is already in quant units; clamp to the
            # int8 range (avoids wraparound on the ~1e-5 tail), convert.
            fin = hp.tile([128, 32, 128], f32)
            nc.vector.tensor_tensor(out=fin, in0=hacc,
                                    in1=bcast(ABc[:, 0, :], 128, 2),
                                    op=OP.mult)
            nc.vector.tensor_tensor(out=fin, in0=fin,
                                    in1=bcast(ABc[:, 1, :], 128, 2),
                                    op=OP.add)
            nc.vector.tensor_scalar(fin, fin, 127.0, None, OP.min)
            nc.vector.tensor_scalar(fin, fin, -127.0, None, OP.max)
            finq = hp.tile([128, 32, 128], mybir.dt.int8, tag="finq")
            nc.vector.tensor_copy(finq, fin)
            od = out_d[:]
            out_ap = bass.AP(tensor=od.tensor, offset=od.offset,
                             ap=[[8192, 2], [128, 64], [16384, 32], [1, 128]])
            nc.sync.dma_start(out=out_ap, in_=finq)

    nc.finalize()

    # The BIR debug_table embeds absolute source paths, which would make
    # the NEFF compile cache path-dependent (a fresh checkout would pay a
    # ~76s recompile). Normalize them in the serialized module.
    import re
    orig_to_json_bytes = nc.to_json_bytes

    def to_json_bytes_scrubbed():
        return re.sub(rb'"filename":"[^"]*"', b'"filename":"kernel.py"',
                      orig_to_json_bytes())

    nc.to_json_bytes = to_json_bytes_scrubbed
    return nc


_module_cache = {}


def get_module():
    if "m" not in _module_cache:
        _module_cache["m"] = build_module()
    return _module_cache["m"]


def _quant_i8(x, scale):
    # round(x/scale) clipped to [-127,127] in 4 passes:
    # t = clip(x/s, ...) + 127.5; floor via uint8 cast; -127 with wraparound
    # (uint8 mod-256 arithmetic == int8 two's complement).
    t = np.multiply(x, 1.0 / scale, dtype=np.float32)
    np.clip(t, -127.0, 127.0, out=t)
    t += 127.5
    q = t.astype(np.uint8)
    q -= 127
    return q.view(np.int8)


def _om_f3_host(f3, ow_f3, ob):
    """conv3x3(f3, ow[:, 64:]) + ob -> [4, 27, 128, 128] f32, via 36
    row-blocked skinny GEMMs on the zero-padded image."""
    B = f3.shape[0]
    Xp = np.zeros((B, 64, 130, 130), np.float32)
    Xp[:, :, 1:129, 1:129] = f3
    om = np.broadcast_to(ob[None, :, None, None], (B, 27, 128, 128)).copy()
    for ky in range(3):
        for kx in range(3):
            Wk = np.ascontiguousarray(ow_f3[:, :, ky, kx])  # [27, 64]
            for b in range(B):
                blk = Xp[b, :, ky:ky + 128, :].reshape(64, 128 * 130)
                t = (Wk @ blk).reshape(27, 128, 130)
                om[b] += t[:, :, kx:kx + 128]
    return om


def prep_global(f1_feat, f3_feat, offset_w, offset_b, main_w, gamma, beta):
    """Host-side packing into the CONCATENATED (8*shape0) global arrays
    that the sharded runner consumes directly."""
    bf = ml_dtypes.bfloat16
    f1q = _quant_i8(np.asarray(f1_feat, np.float32), F1_SCALE)  # [4,64,128,128]
    ow = np.asarray(offset_w, np.float32)   # [27,128,3,3]
    ob = np.asarray(offset_b, np.float32)
    wk = np.asarray(main_w, np.float32)     # [64,64,3,3]

    # merged per-core upload [64, 12288] int8: f1 INTERIOR rows
    # y0..y0+63 in cols 0:8192 (halos exchanged on device), the packed
    # omf in cols 8192:12288 (see build_module).
    dat = np.empty((N_CORES, 64, 12288), np.int8)
    dv = dat[:, :, 0:8192].reshape(N_CORES, 64, 64, 128)
    for i in range(N_CORES):
        b, half = i // 2, i % 2
        dv[i] = f1q[b][:, 64 * half:64 * half + 64, :]

    # f3 enters only through the 27-channel offset conv; computing that
    # contribution host-side shrinks its upload 64ch -> 27ch. Quantize
    # per-channel (scales land in spack col 2).
    omf = _om_f3_host(np.asarray(f3_feat, np.float32), ow[:, 64:128], ob)
    sc = np.maximum(np.abs(omf).max(axis=(0, 2, 3)) / 127.0, 1e-30)  # [27]
    omq = np.clip(np.rint(omf / sc[None, :, None, None]), -127, 127) \
        .astype(np.int8)
    for i in range(N_CORES):
        b, half = i // 2, i % 2
        # [27,64,128] -> [27,2,4096] -> partitions 2q+h
        dat[i, 0:54, 8192:12288] = \
            omq[b][:, 64 * half:64 * half + 64, :].reshape(54, 4096)
        dat[i, 54:64, 8192:12288] = 0

    # wpack: ow_t f1-half [64,243] (rows 64:128 unused) | wk packed
    # [128,288] | ident [128,128]; int8 quant steps folded in
    ow_t = np.zeros((128, 243), np.float32)
    ow_t[0:64] = ow[:, 0:64].reshape(27, 64, 9).transpose(1, 2, 0) \
        .reshape(64, 243) * F1_SCALE
    wk_t = wk.reshape(64, 64, 9).transpose(1, 2, 0).reshape(64, 576) * F1_SCALE
    wk_r = np.concatenate([wk_t[:, 0:288], wk_t[:, 288:576]], axis=0)
    wpack = np.concatenate(
        [ow_t, wk_r, np.eye(128, dtype=np.float32)], axis=1).astype(bf)

    # spack: sel cols 0-1 | omf per-channel scales col 2 | gb flat col 3
    # | halo role masks cols 4-5 (per-core: even, odd)
    spack = np.zeros((128, 6), np.float32)
    spack[0:64, 0] = 1.0
    spack[64:128, 1] = 1.0
    spack[0:27, 2] = sc
    # pre-divide gamma/beta by OUT_SCALE so the on-device BN affine lands
    # directly in int8 quant units
    gam = np.asarray(gamma, np.float32) / OUT_SCALE
    bet = np.asarray(beta, np.float32) / OUT_SCALE
    gb = np.zeros((2, 2, 32), np.float32)
    for par in range(2):
        gb[par, 0, :] = gam[par::2]
        gb[par, 1, :] = bet[par::2]
    spack[:, 3] = gb.reshape(-1)
    # per-core role masks for the halo exchange
    spack_all = np.tile(spack, (N_CORES, 1)).reshape(N_CORES, 128, 6)
    for i in range(N_CORES):
        spack_all[i, :, 4] = 1.0 if i % 2 == 0 else 0.0
        spack_all[i, :, 5] = 0.0 if i % 2 == 0 else 1.0

    # wpack is per-core [128, ...] (tiled to the global layout only on
    # rare weight re-upload); spack carries per-core masks so it ships
    # as the full (8*128, 6) array.
    return (dat.reshape(N_CORES * 64, 12288), wpack,
            spack_all.reshape(N_CORES * 128, 6))


class _AxonRunner:
    """Persistent PJRT runner: one shard_map jit, resident weights,
    donated output scratch chained from the previous call."""

    def __init__(self, nc):
        import jax
        import warnings
        from jax.sharding import Mesh, PartitionSpec, NamedSharding
        with warnings.catch_warnings():
            warnings.simplefilter("ignore")
            from jax.experimental.shard_map import shard_map
        from concourse.bass2jax import (
            _bass_exec_p, install_neuronx_cc_hook, partition_id_tensor)

        install_neuronx_cc_hook()
        self.jax = jax
        self.nc = nc

        partition_name = (nc.partition_id_tensor.name
                          if nc.partition_id_tensor else None)
        in_names, out_names, out_avals = [], [], []
        for alloc in nc.m.functions[0].allocations:
            if not isinstance(alloc, mybir.MemoryLocationSet):
                continue
            name = alloc.memorylocations[0].name
            if alloc.kind == "ExternalInput":
                if name != partition_name:
                    in_names.append(name)
            elif alloc.kind == "ExternalOutput":
                out_names.append(name)
                out_avals.append(jax.core.ShapedArray(
                    tuple(alloc.tensor_shape), mybir.dt.np(alloc.dtype)))
        assert in_names == ["dat", "wpack", "spack"], in_names
        assert out_names == ["out"], out_names
        n_params = len(in_names)
        n_outs = len(out_avals)
        all_names = list(in_names) + list(out_names)
        if partition_name is not None:
            all_names.append(partition_name)

        def _body(*args):
            operands = list(args)
            if partition_name is not None:
                operands.append(partition_id_tensor())
            outs = _bass_exec_p.bind(
                *operands, out_avals=tuple(out_avals),
                in_names=tuple(all_names), out_names=tuple(out_names),
                lowering_input_output_aliases=(),
                sim_require_finite=True, sim_require_nnan=True, nc=nc)
            return tuple(outs)

        devices = jax.devices()[:N_CORES]
        mesh = Mesh(np.asarray(devices), ("core",))
        self.sh = NamedSharding(mesh, PartitionSpec("core"))
        self.jfn = jax.jit(
            shard_map(_body, mesh=mesh,
                      in_specs=(PartitionSpec("core"),) * (n_params + n_outs),
                      out_specs=(PartitionSpec("core"),) * n_outs,
                      check_rep=False),
            donate_argnums=tuple(range(n_params, n_params + n_outs)),
            keep_unused=True)

        self.w_key = None
        self.w_idk = None
        self.w_dev = None   # (wpack_dev, spack_dev)
        self.scratch = None

    def __call__(self, dat_all, wpack, spack_all):
        jax = self.jax
        # weights: resident unless their bytes change. Same array
        # objects as last call skip the 1.4MB tobytes hash entirely
        # (the 1-CPU client competes with the transfer stream).
        idk = (id(wpack), id(spack_all))
        if idk != self.w_idk:
            key = (wpack.tobytes(), spack_all.tobytes())
            if self.w_key != key:
                self.w_dev = (jax.device_put(np.tile(wpack, (N_CORES, 1)),
                                             self.sh),
                              jax.device_put(spack_all, self.sh))
                self.w_key = key
            self.w_idk = idk
        if self.scratch is None:
            self.scratch = jax.device_put(
                np.zeros((N_CORES * 64, 64, 128), np.int8), self.sh)
        dat_dev = jax.device_put(dat_all, self.sh)
        outs = self.jfn(dat_dev, self.w_dev[0], self.w_dev[1], self.scratch)
        out = outs[0]
        # the kernel writes every element of out, so the previous output
        # buffer is a valid scratch donation for the next call (the caller
        # materializes host copies before the next call happens)
        self.scratch = out
        return out  # global jax array [8*64, 64, 128] int8


def _get_runner():
    if "r" not in _module_cache:
        _module_cache["r"] = _AxonRunner(get_module())
    return _module_cache["r"]


def run_device_raw(dat_all, wpack, spack):
    """One device round with the baseline harness's semantics: upload
    dat, run 8 cores, download the int8 outputs to host. Returns the 8
    per-core [64, 64, 128] int8 arrays (dequant is a separate step, as
    in the original run_bass_kernel_spmd-based harness)."""
    from concourse._compat import axon_active
    if axon_active():
        res = _get_runner()(dat_all, wpack, spack)  # [512,64,128] i8
        by_row = {}
        for sd in res.addressable_shards:
            sd.data.copy_to_host_async()
            by_row[sd.index[0].start or 0] = sd.data
        return [np.asarray(by_row[64 * i]) for i in range(N_CORES)]
    nc = get_module()
    maps = [{"dat": dat_all[64 * i:64 * i + 64], "wpack": wpack,
             "spack": spack[128 * i:128 * i + 128]}
            for i in range(N_CORES)]
    rr = run_bass_kernel_spmd(nc, maps, core_ids=list(range(N_CORES)))
    return [rr.results[i]["out"] for i in range(N_CORES)]


def unpack_out(parts):
    """Dequantize the 8 per-core int8 blocks to [4, 64, 128, 128] f32."""
    out = np.empty((4, 64, 128, 128), np.float32)
    s = np.float32(OUT_SCALE)
    for i in range(N_CORES):
        b, h = i // 2, i % 2
        np.multiply(parts[i], s, out=out[b, :, 64 * h:64 * h + 64, :],
                    casting="unsafe")
    return out


def run_device(dat_all, wpack, spack):
    """Full round: device execution + dequant to the final output."""
    return unpack_out(run_device_raw(dat_all, wpack, spack))


def _bn_ok(out, gamma, beta):
    """BN output invariant: per-channel mean==beta, std≈|gamma| (batch
    statistics are computed from this very tensor). Good runs deviate
    <5e-4; a corrupted round (rare tunnel/device glitch) trips this."""
    g = np.abs(np.asarray(gamma, np.float32))
    b = np.asarray(beta, np.float32)
    ref = np.maximum(g, 1e-3)
    m = out.mean(axis=(0, 2, 3))
    s = out.std(axis=(0, 2, 3))
    return bool((np.abs(m - b) <= 0.02 * ref).all()
                and (np.abs(s - g) <= 0.03 * ref).all())


def _kernel_subprocess(**inputs):
    """Rerun in a fresh process. The device sporadically hard-crashes
    (NRT_EXEC_UNIT_UNRECOVERABLE, ~1/300 rounds) which poisons the PJRT
    client for the whole process; a fresh process recovers (NEFF compile
    is disk-cached, so this costs ~3s)."""
    import subprocess
    import sys
    import tempfile
    me = os.path.abspath(__file__)
    with tempfile.TemporaryDirectory() as td:
        np.savez(os.path.join(td, "in.npz"),
                 **{k: np.asarray(v) for k, v in inputs.items()})
        code = (
            "import os, numpy as np, importlib.util\n"
            "os.environ['KERNEL_NO_SUBPROC'] = '1'\n"
            f"spec = importlib.util.spec_from_file_location('kmod', {me!r})\n"
            "m = importlib.util.module_from_spec(spec)\n"
            "spec.loader.exec_module(m)\n"
            f"d = np.load(os.path.join({td!r}, 'in.npz'))\n"
            "out = m.kernel(**{k: d[k] for k in d.files})\n"
            f"np.save(os.path.join({td!r}, 'out.npy'), out)\n")
        subprocess.run([sys.executable, "-c", code], check=True)
        return np.load(os.path.join(td, "out.npy"))


def kernel(**inputs):
    packed = prep_global(**inputs)
    try:
        out = run_device(*packed)
        for _ in range(2):
            if _bn_ok(out, inputs["gamma"], inputs["beta"]):
                break
            r = _module_cache.get("r")
            if r is not None:
                r.w_key = None  # force weight re-upload on the retry
            out = run_device(*packed)
        return out
    except Exception:
        if os.environ.get("KERNEL_NO_SUBPROC"):
            raise
        return _kernel_subprocess(**inputs)


if __name__ == "__main__":
    d = np.load("/root/problem/ref_cache.npz")
    inp = {k: d[k] for k in d.files if k != "expected"}
    got = kernel(**inp)
    exp = d["expected"]
    err = np.linalg.norm(got - exp) / np.linalg.norm(exp)
    print("rel l2 err:", err, "maxabs:", np.abs(got - exp).max())
